# revision 1
# baseline (speedup 1.0000x reference)
"""CrossNetMix (moe_routing) Trainium2 Bass kernel.

Math (per layer i, softmax gates g sum to 1 over E):
    x_{l+1} = x_l + x0 * (sum_e g_e * U_e @ tanh(C_e @ tanh(V_e^T x_l)) + bias_i)

Key transform: the residual chain collapses to
    x_L = x0 * (1 + sum_i (acc_i + bias_i))     with acc_i the gated MoE out,
so we carry u_i = 1 + sum_{j<i} (acc_j + bias_j) and materialize
y_i = y0 * u_i (transposed space y = x^T [D, B]) only as matmul input.

Per layer (all matmuls contract over partitions, everything transposed):
  - S0 gating:  glog[4,B]  = G^T-chunks (lhsT) x y_i        (8 K-chunks, PSUM)
  - softmax:    eg = exp(glog); Z4 = ones44 x eg; rZ ~ 1/Z; gn4 = eg*rZ
  - broadcast:  gbc[256,B] = Sel x gn4                      (2 matmuls)
  - S1 V-stage: v[256,B]   = packed-V-pairs (lhsT) x y_i    (2x8 matmuls, PSUM)
  - S2 C-stage: w[256,B]   = blockdiag-C^T x tanh(v)        (2 matmuls)
  - wg = tanh(w) * gbc
  - S3 U-stage: acc[1024,B] = packed-U x wg                 (8 M x 2 K matmuls)
  - u update:   layer0: u = acc + (1+bias) on ACT (PSUM evac w/ free bias)
                layer1+: u += acc (+bias) in-place on DVE
  - y_{i+1} = y0 * u  (SBUF-only mul, split DVE/GPSIMD; final one is output)

Matmul operands are float32r (full-rate PE, ~1e-4 matmul accuracy); the
carried u accumulator stays fp32 and the final output is computed in fp32.
B=16384 sharded over 8 cores (2048 each), 4 chunks of 512 columns.
"""

import numpy as np

B, D, R, E, L = 16384, 1024, 64, 4, 3
NCORES = 8
BC = B // NCORES            # columns per core
BT = 512                    # columns per chunk (= fp32 PSUM bank capacity)
NCHUNK = BC // BT
KC = D // 128               # K-chunks over D
NM = D // 128               # M-chunks over D

# y = y0*u materialization steps j=1..3 and m-chunks routed to GPSIMD
MUL_ON_GPSIMD = {(j, m) for j in (1, 2, 3) for m in range(NM) if m % 3 != 2}

_CACHE = {}


def _build(bias_nonzero: bool, reps: int = 1, cfg: dict | None = None):
    import concourse.mybir as mybir
    import concourse.bacc as bacc
    import concourse.tile as tile

    cfg = cfg or {}
    psg, psgbc, psvw, psacc = cfg.get("psum", (1, 2, 2, 3))
    y0b, ub, yb, tb = cfg.get("sbuf", (3, 2, 3, 2))
    splity0 = cfg.get("splity0", True)
    glogpack = cfg.get("glogpack", False)
    nsplit = cfg.get("nsplit", False)
    gpsn = cfg.get("gpsn", None)  # muls per layer on gpsimd (None -> MUL_ON_GPSIMD)
    mul_gps = (MUL_ON_GPSIMD if gpsn is None else
               {(j, m) for j in (1, 2, 3) for m in range(NM) if m < gpsn})

    f32 = mybir.dt.float32
    f32r = mybir.dt.float32r
    ALU = mybir.AluOpType
    ACTF = mybir.ActivationFunctionType

    nc = bacc.Bacc("TRN2", target_bir_lowering=False, debug=False,
                   num_devices=NCORES)

    xT = nc.dram_tensor("xT", [KC, 128, BC], f32r, kind="ExternalInput")
    GT = nc.dram_tensor("GT", [KC, 128, E], f32r, kind="ExternalInput")
    GTP = nc.dram_tensor("GTP", [KC, 128, 64], f32r, kind="ExternalInput")
    SEL16 = nc.dram_tensor("SEL16", [128, E], f32r, kind="ExternalInput")
    VP = nc.dram_tensor("VP", [L, KC, 128, 2, 128], f32r, kind="ExternalInput")
    CB = nc.dram_tensor("CB", [L, 2, 128, 128], f32r, kind="ExternalInput")
    UP = nc.dram_tensor("UP", [L, 2, 128, NM, 128], f32r, kind="ExternalInput")
    SEL = nc.dram_tensor("SEL", [E, 2, 128], f32r, kind="ExternalInput")
    ONES = nc.dram_tensor("ONES", [E, E], f32r, kind="ExternalInput")
    # BIA[:, i*NM+m] = bias[i, m*128:(m+1)*128] (+1.0 folded in for i==0)
    BIA = nc.dram_tensor("BIA", [128, L * NM], f32, kind="ExternalInput")
    outT = nc.dram_tensor("outT", [KC, 128, BC], f32, kind="ExternalOutput")

    with tile.TileContext(nc) as tc:
        with (
            tc.tile_pool(name="wts", bufs=1) as wts,
            tc.tile_pool(name="y0p", bufs=y0b) as y0p,
            tc.tile_pool(name="yp", bufs=yb) as yp,
            tc.tile_pool(name="up", bufs=ub) as upool,
            tc.tile_pool(name="tp", bufs=tb) as tp,
            tc.tile_pool(name="twp", bufs=tb) as twp,
            tc.tile_pool(name="wgp", bufs=tb) as wgp,
            tc.tile_pool(name="gp", bufs=cfg.get("gpb", 2)) as gp,
            tc.tile_pool(name="ps_g", bufs=psg, space="PSUM") as ps_g,
            tc.tile_pool(name="ps_gbc", bufs=psgbc, space="PSUM") as ps_gbc,
            tc.tile_pool(name="ps_vw", bufs=psvw, space="PSUM") as ps_vw,
            tc.tile_pool(name="ps_acc", bufs=psacc, space="PSUM") as ps_acc,
        ):
            # ---- weights: layer-0 + small tensors first so PE starts early
            if glogpack:
                gt_sb = wts.tile([128, KC, 64], f32r, tag="gt")
                nc.sync.dma_start(out=gt_sb[:],
                                  in_=GTP.rearrange("kc p e -> p kc e"))
                sel16_sb = wts.tile([128, E], f32r, tag="sel16")
                nc.sync.dma_start(out=sel16_sb[:], in_=SEL16[:, :])
            else:
                gt_sb = wts.tile([128, KC, E], f32r, tag="gt")
                nc.sync.dma_start(out=gt_sb[:],
                                  in_=GT.rearrange("kc p e -> p kc e"))
            sel_sb = wts.tile([E, 2, 128], f32r, tag="sel")
            nc.sync.dma_start(out=sel_sb[:], in_=SEL[:, :, :])
            ones_sb = wts.tile([E, E], f32r, tag="ones")
            nc.sync.dma_start(out=ones_sb[:], in_=ONES[:, :])
            bia_sb = wts.tile([128, L * NM], f32, tag="bia")
            nc.sync.dma_start(out=bia_sb[:], in_=BIA[:, :])
            vp_sb, cb_sb, up_sb = [], [], []
            for i in range(L):
                vp_sb.append(wts.tile([128, KC, 2, 128], f32r, tag=f"vp{i}",
                                      name=f"vp{i}"))
                cb_sb.append(wts.tile([128, 2, 128], f32r, tag=f"cb{i}",
                                      name=f"cb{i}"))
                up_sb.append(wts.tile([128, 2, NM, 128], f32r, tag=f"up{i}",
                                      name=f"up{i}"))

            def load_layer_weights(i, split=False):
                if split:
                    for kk in range(KC):
                        nc.sync.dma_start(
                            out=vp_sb[i][:, kk, :, :], in_=VP[i, kk])
                else:
                    nc.sync.dma_start(
                        out=vp_sb[i][:],
                        in_=VP[i].rearrange("kc p pr m -> p kc pr m"))
                nc.sync.dma_start(out=cb_sb[i][:],
                                  in_=CB[i].rearrange("k2 p m -> p k2 m"))
                nc.sync.dma_start(out=up_sb[i][:],
                                  in_=UP[i].rearrange("k2 p mc m -> p k2 mc m"))

            load_layer_weights(0)

            state = {}

            def load_chunk(cidx, rep):
                c0 = (cidx % NCHUNK) * BT
                y0 = y0p.tile([128, KC, BT], f32r, tag="y0",
                              name=f"y0_{rep}_{cidx}")
                if splity0:
                    for kk in range(KC):
                        nc.sync.dma_start(out=y0[:, kk, :],
                                          in_=xT[kk, :, c0:c0 + BT])
                else:
                    nc.sync.dma_start(
                        out=y0[:],
                        in_=xT[:, :, c0:c0 + BT].rearrange("kc p b -> p kc b"))
                state[cidx] = {"y0": y0, "y_in": y0, "u": None}

            def emit_layer(cidx, i):
                st = state[cidx]
                y0, y_in = st["y0"], st["y_in"]
                if i == 0:
                    st["u"] = upool.tile([128, NM, BT], f32, tag="u",
                                         name=f"u_{cidx}")
                u = st["u"]
                # --- S1 V-stage ---
                v_ps = [ps_vw.tile([128, BT], f32, tag="vw",
                                   name=f"v{pr_}") for pr_ in range(2)]
                nh = 2 if nsplit else 1
                hw_ = BT // nh
                for pr in range(2):
                    for h in range(nh):
                        hs = slice(h * hw_, (h + 1) * hw_)
                        for k in range(KC):
                            nc.tensor.matmul(
                                v_ps[pr][:, hs], vp_sb[i][:, k, pr, :],
                                y_in[:, k, hs],
                                start=(k == 0), stop=(k == KC - 1))
                # --- S0 gating logits ---
                if glogpack:
                    # 2-way col-tiled partials (fp32 xbus budget: bases 0/64)
                    pg = ps_g.tile([128, BT], f32, tag="g", name="pg")
                    for idx, k in enumerate(range(KC)):
                        j = 64 * (k % 2)
                        nc.tensor.matmul(
                            pg[j:j + 64, :], gt_sb[:, k, :],
                            y_in[:, k, :],
                            start=(idx == 0), stop=(idx == KC - 1),
                            tile_position=(0, j))
                    pg_sb = gp.tile([128, BT], f32r, tag="pg_sb", bufs=1)
                    nc.scalar.activation(pg_sb[:], pg[:], ACTF.Copy, bias=0.0)
                    glog = ps_g.tile([E, BT], f32, tag="g")
                    nc.tensor.matmul(glog[:], sel16_sb[:], pg_sb[:],
                                     start=True, stop=True)
                else:
                    glog = ps_g.tile([E, BT], f32, tag="g")
                    for k in range(KC):
                        nc.tensor.matmul(glog[:], gt_sb[:, k, :], y_in[:, k, :],
                                         start=(k == 0), stop=(k == KC - 1))
                t_sb = [tp.tile([128, BT], f32r, tag="t",
                                name=f"t{pr_}") for pr_ in range(2)]
                for pr in range(2):
                    for h in range(nh):
                        hs = slice(h * hw_, (h + 1) * hw_)
                        nc.scalar.activation(t_sb[pr][:, hs], v_ps[pr][:, hs],
                                             ACTF.Tanh)
                eg = gp.tile([E, BT], f32r, tag="eg")
                nc.scalar.activation(eg[:], glog[:], ACTF.Exp)
                z4 = ps_g.tile([E, BT], f32, tag="g")
                nc.tensor.matmul(z4[:], ones_sb[:], eg[:], start=True, stop=True)
                rz4 = gp.tile([E, BT], f32, tag="rz", bufs=1)
                nc.vector.reciprocal_approx_fast(out=rz4[:], in_=z4[:])
                gn4 = gp.tile([E, BT], f32r, tag="gn")
                nc.vector.tensor_mul(out=gn4[:], in0=eg[:], in1=rz4[:])
                # --- S2 C-stage ---
                w_ps = [ps_vw.tile([128, BT], f32, tag="vw",
                                   name=f"w{pr_}") for pr_ in range(2)]
                for pr in range(2):
                    for h in range(nh):
                        hs = slice(h * hw_, (h + 1) * hw_)
                        nc.tensor.matmul(w_ps[pr][:, hs], cb_sb[i][:, pr, :],
                                         t_sb[pr][:, hs],
                                         start=(h == 0), stop=(h == nh - 1))
                gbc_ps = [ps_gbc.tile([128, BT], f32, tag="gbc",
                                      name=f"gbc{pr_}") for pr_ in range(2)]
                for pr in range(2):
                    nc.tensor.matmul(gbc_ps[pr][:], sel_sb[:, pr, :],
                                     gn4[:], start=True, stop=True)
                wg_sb = []
                for pr in range(2):
                    tw = twp.tile([128, BT], f32, tag="tw")
                    for h in range(nh):
                        hs = slice(h * hw_, (h + 1) * hw_)
                        nc.scalar.activation(tw[:, hs], w_ps[pr][:, hs],
                                             ACTF.Tanh)
                    wg = wgp.tile([128, BT], f32r, tag="wg")
                    nc.vector.tensor_mul(out=wg[:], in0=tw[:],
                                         in1=gbc_ps[pr][:])
                    wg_sb.append(wg)
                # --- S3 U-stage + u update + y materialization ---
                last = (i == L - 1)
                y_out = yp.tile([128, KC, BT], f32 if last else f32r, tag="y")
                for m in range(NM):
                    acc = ps_acc.tile([128, BT], f32, tag="acc")
                    nc.tensor.matmul(acc[:], up_sb[i][:, 0, m, :],
                                     wg_sb[0][:], start=True, stop=False)
                    nc.tensor.matmul(acc[:], up_sb[i][:, 1, m, :],
                                     wg_sb[1][:], start=False, stop=True)
                    bcol = bia_sb[:, i * NM + m: i * NM + m + 1]
                    if i == 0:
                        if bias_nonzero:
                            nc.scalar.activation(u[:, m, :], acc[:],
                                                 ACTF.Identity, bias=bcol)
                        else:
                            nc.scalar.activation(u[:, m, :], acc[:],
                                                 ACTF.Copy, bias=1.0)
                    else:
                        if bias_nonzero:
                            nc.vector.scalar_tensor_tensor(
                                out=u[:, m, :], in0=acc[:], scalar=bcol,
                                in1=u[:, m, :], op0=ALU.add, op1=ALU.add)
                        else:
                            nc.vector.tensor_add(out=u[:, m, :], in0=acc[:],
                                                 in1=u[:, m, :])
                    eng = (nc.gpsimd if (i + 1, m) in mul_gps
                           else nc.vector)
                    eng.tensor_mul(out=y_out[:, m, :], in0=y0[:, m, :],
                                   in1=u[:, m, :])
                st["y_in"] = y_out
                if last:
                    c0 = (cidx % NCHUNK) * BT
                    if splity0:
                        for kk in range(KC):
                            nc.sync.dma_start(out=outT[kk, :, c0:c0 + BT],
                                              in_=y_out[:, kk, :])
                    else:
                        nc.sync.dma_start(
                            out=outT[:, :, c0:c0 + BT].rearrange(
                                "kc p b -> p kc b"),
                            in_=y_out[:])

            # software-pipelined emission: per step emit L0(c), L2(c-1), L1(c)
            total = reps * NCHUNK
            for gc in range(total + 1):
                if gc < total:
                    load_chunk(gc, 0)
                    if gc == 0:
                        load_layer_weights(1)
                        load_layer_weights(2)
                    emit_layer(gc, 0)
                if gc >= 1:
                    emit_layer(gc - 1, 2)
                    del state[gc - 1]
                if gc < total:
                    emit_layer(gc, 1)
    nc.compile()
    return nc


def _get_nc(bias_nonzero: bool):
    key = ("nc", bias_nonzero)
    if key not in _CACHE:
        _CACHE[key] = _build(bias_nonzero)
    return _CACHE[key]


def _prep_inputs(inputs, U, V, C, G, bias):
    """Host-side layout prep -> per-core in_maps."""
    f32 = np.float32
    xT = np.ascontiguousarray(inputs.T.astype(f32, copy=False))  # [D, B]
    GTh = np.ascontiguousarray(G.T).reshape(KC, 128, E).astype(f32, copy=False)
    GTPh = np.zeros((KC, 128, 64), f32)
    GTPh[:, :, :E] = GTh
    SEL16h = np.zeros((128, E), f32)
    for p in range(128):
        if p % 64 < E:
            SEL16h[p, p % 64] = 1.0
    VPh = np.ascontiguousarray(
        V.transpose(0, 2, 1, 3).reshape(L, D, E * R).reshape(L, KC, 128, 2, 128))
    CBh = np.zeros((L, 2, 128, 128), f32)
    for i in range(L):
        for pr in range(2):
            CBh[i, pr, :64, :64] = C[i, 2 * pr].T
            CBh[i, pr, 64:, 64:] = C[i, 2 * pr + 1].T
    UPh = np.ascontiguousarray(
        U.transpose(0, 1, 3, 2).reshape(L, E * R, D).reshape(L, 2, 128, NM, 128))
    SELh = np.zeros((E, 2 * 128), f32)
    for e in range(E):
        SELh[e, e * 64:(e + 1) * 64] = 1.0
    SELh = SELh.reshape(E, 2, 128)
    ONESh = np.ones((E, E), f32)
    biasm = bias.astype(f32, copy=True)
    biasm[0] += 1.0       # fold the residual "1 +" into layer-0 bias
    BIAh = np.ascontiguousarray(
        biasm.reshape(L, NM, 128).transpose(2, 0, 1).reshape(128, L * NM))

    in_maps = []
    for c in range(NCORES):
        xTc = np.ascontiguousarray(
            xT[:, c * BC:(c + 1) * BC]).reshape(KC, 128, BC)
        in_maps.append({
            "xT": xTc, "GT": GTh, "GTP": GTPh, "SEL16": SEL16h,
            "VP": VPh, "CB": CBh, "UP": UPh,
            "SEL": SELh, "ONES": ONESh, "BIA": BIAh,
        })
    return in_maps


def kernel(inputs, U, V, C, G, bias):
    from concourse.bass_utils import run_bass_kernel_spmd

    inputs = np.asarray(inputs, dtype=np.float32)
    U = np.asarray(U, dtype=np.float32)
    V = np.asarray(V, dtype=np.float32)
    C = np.asarray(C, dtype=np.float32)
    G = np.asarray(G, dtype=np.float32)
    bias = np.asarray(bias, dtype=np.float32)

    bias_nonzero = bool(np.any(bias != 0.0))
    nc = _get_nc(bias_nonzero)
    in_maps = _prep_inputs(inputs, U, V, C, G, bias)
    res = run_bass_kernel_spmd(nc, in_maps, core_ids=list(range(NCORES)))
    out = np.empty((D, B), np.float32)
    for c in range(NCORES):
        out[:, c * BC:(c + 1) * BC] = res.results[c]["outT"].reshape(D, BC)
    return np.ascontiguousarray(out.T)



# revision 6
# speedup vs baseline: 2.2182x; 2.2182x over previous
"""CrossNetMix (moe_routing) Trainium2 Bass kernel — transfer-optimized.

Math (per layer i, softmax gates g sum to 1 over E):
    x_{l+1} = x_l + x0 * (sum_e g_e * U_e @ tanh(C_e @ tanh(V_e^T x_l)) + bias_i)

The residual chain collapses to x_L = x0 * (1 + sum_i (acc_i + bias_i)), so we
carry u_i = 1 + sum_{j<i} (acc_j + bias_j) and materialize y_i = y0 * u_i
(transposed space y = x^T) only as matmul input.

The axon host<->device tunnel runs at ~40 MB/s, so wall time is dominated by
transfer bytes, not compute (~0.2 ms of HW work). Transfer plan:
  - x uploaded as fp16 in natural [B, D] row layout (no host transpose);
    transposed on device via PE-identity matmuls.
  - all weights packed host-side into one fp16 blob [128, 13112], sharded
    8-ways by rows (each core uploads 1/8 = 0.42 MB) and reassembled on
    device with a NeuronLink AllGather.
  - constant matrices (identity / expert-select / ones) ride inside the NEFF
    via inline_tensor — zero upload.
  - output produced as fp16 [B, D] rows (device-side transpose back).
Gating softmax stays in fp32 (fp16 exp would overflow for |logit| > 11);
the u accumulator stays fp32; everything else computes from fp16 operands
with fp32 PSUM accumulation (~1e-3 matmul accuracy).
B=16384 sharded over 8 cores (2048 rows each), 4 chunks of 512 batch cols.
"""

import numpy as np

B, D, R, E, L = 16384, 1024, 64, 4, 3
NCORES = 8
BC = B // NCORES            # batch rows per core
BT = 512                    # batch columns per chunk (fp32 PSUM bank capacity)
NCHUNK = BC // BT
KC = D // 128               # K-chunks over D
NM = D // 128               # M-chunks over D

# packed fp16 weight blob [128, WCOLS] column offsets
OFF_GT = 0                  # [128, KC*E]        gating weights
OFF_BIA = OFF_GT + KC * E   # [128, L*NM]        bias (+1 folded into layer 0)
OFF_VP = OFF_BIA + L * NM   # L x [128, KC*2*128]
OFF_CB = OFF_VP + L * KC * 2 * 128   # L x [128, 2*128]
OFF_UP = OFF_CB + L * 2 * 128        # L x [128, 2*NM*128]
WCOLS = OFF_UP + L * 2 * NM * 128
WROWS_SH = 128 // NCORES    # blob rows uploaded per core

_CACHE = {}


def _build():
    import concourse.mybir as mybir
    import concourse.bacc as bacc
    import concourse.tile as tile

    f32 = mybir.dt.float32
    f16 = mybir.dt.float16
    ALU = mybir.AluOpType
    ACTF = mybir.ActivationFunctionType

    nc = bacc.Bacc("TRN2", target_bir_lowering=False, debug=False,
                   num_devices=NCORES)

    X = nc.dram_tensor("X", [BC, D], f16, kind="ExternalInput")
    WSH = nc.dram_tensor("WSH", [WROWS_SH, WCOLS], f16, kind="ExternalInput")
    OUT = nc.dram_tensor("OUT", [BC, D], f16, kind="ExternalOutput")

    wbnc = nc.dram_tensor("wbnc", [WROWS_SH, WCOLS], f16)
    wall = nc.dram_tensor("wall", [128, WCOLS], f16, addr_space="Shared")

    # inline constants (shipped inside the NEFF, no upload)
    ident_h = np.eye(128, dtype=np.float16)
    sel_h = np.zeros((E, 2, 128), np.float32)
    for e in range(E):
        sel_h.reshape(E, 256)[e, e * 64:(e + 1) * 64] = 1.0
    ones_h = np.ones((E, E), np.float32)

    with tile.TileContext(nc) as tc:
        with (
            tc.tile_pool(name="wts", bufs=1) as wts,
            tc.tile_pool(name="xrp", bufs=8) as xrp,
            tc.tile_pool(name="y0p", bufs=3) as y0p,
            tc.tile_pool(name="yp", bufs=3) as yp,
            tc.tile_pool(name="up", bufs=2) as upool,
            tc.tile_pool(name="tp", bufs=2) as tp,
            tc.tile_pool(name="twp", bufs=2) as twp,
            tc.tile_pool(name="wgp", bufs=2) as wgp,
            tc.tile_pool(name="gp", bufs=2) as gp,
            tc.tile_pool(name="orp", bufs=2) as orp,
            tc.tile_pool(name="ps_g", bufs=1, space="PSUM") as ps_g,
            tc.tile_pool(name="ps_gbc", bufs=2, space="PSUM") as ps_gbc,
            tc.tile_pool(name="ps_vw", bufs=2, space="PSUM") as ps_vw,
            tc.tile_pool(name="ps_acc", bufs=2, space="PSUM") as ps_acc,
            tc.tile_pool(name="ps_tr", bufs=1, space="PSUM") as ps_tr,
        ):
            # gather the 8 weight shards over NeuronLink ASAP
            nc.sync.dma_start(out=wbnc[:, :], in_=WSH[:, :])
            nc.gpsimd.collective_compute(
                "AllGather", mybir.AluOpType.bypass,
                replica_groups=[list(range(NCORES))],
                ins=[wbnc.ap()], outs=[wall.ap()])

            id_sb = wts.tile([128, 128], f16, tag="id")
            nc.sync.dma_start(out=id_sb[:], in_=nc.inline_tensor(
                ident_h, name="ident")[:, :])
            sel_sb = wts.tile([E, 2, 128], f32, tag="sel")
            nc.sync.dma_start(out=sel_sb[:], in_=nc.inline_tensor(
                sel_h, name="sel")[:, :, :])
            ones_sb = wts.tile([E, E], f32, tag="ones")
            nc.sync.dma_start(out=ones_sb[:], in_=nc.inline_tensor(
                ones_h, name="ones")[:, :])

            # weight tiles from the gathered blob
            gt_sb = wts.tile([128, KC * E], f16, tag="gt")
            nc.sync.dma_start(out=gt_sb[:],
                              in_=wall[:, OFF_GT:OFF_GT + KC * E])
            bia16 = wts.tile([128, L * NM], f16, tag="bia16")
            nc.sync.dma_start(out=bia16[:],
                              in_=wall[:, OFF_BIA:OFF_BIA + L * NM])
            bia_sb = wts.tile([128, L * NM], f32, tag="bia")
            nc.scalar.activation(bia_sb[:], bia16[:], ACTF.Copy)
            vp_sb, cb_sb, up_sb = [], [], []
            for i in range(L):
                vp = wts.tile([128, KC * 2 * 128], f16, tag=f"vp{i}")
                nc.sync.dma_start(
                    out=vp[:], in_=wall[:, OFF_VP + i * KC * 256:
                                        OFF_VP + (i + 1) * KC * 256])
                vp_sb.append(vp)
                cb = wts.tile([128, 2 * 128], f16, tag=f"cb{i}")
                nc.sync.dma_start(
                    out=cb[:], in_=wall[:, OFF_CB + i * 256:
                                        OFF_CB + (i + 1) * 256])
                cb_sb.append(cb)
                up = wts.tile([128, 2 * NM * 128], f16, tag=f"up{i}")
                nc.sync.dma_start(
                    out=up[:], in_=wall[:, OFF_UP + i * NM * 256:
                                        OFF_UP + (i + 1) * NM * 256])
                up_sb.append(up)

            def load_chunk(cidx):
                """DMA 512 batch rows and transpose to y0 [128, KC, BT]."""
                r0 = cidx * BT
                xr = []
                for bi in range(4):
                    t_ = xrp.tile([128, D], f16, tag=f"xr{bi}")
                    nc.sync.dma_start(
                        out=t_[:], in_=X[r0 + bi * 128:r0 + (bi + 1) * 128, :])
                    xr.append(t_)
                y0 = y0p.tile([128, KC, BT], f16, tag="y0",
                              name=f"y0_{cidx}")
                for dj in range(KC):
                    pt = ps_tr.tile([128, D], f16, tag="tr", name="pt")
                    for bi in range(4):
                        nc.tensor.transpose(
                            pt[:, bi * 128:(bi + 1) * 128],
                            xr[bi][:, dj * 128:(dj + 1) * 128], id_sb[:])
                    nc.scalar.activation(y0[:, dj, :], pt[:, :BT], ACTF.Copy)
                return {"y0": y0, "y_in": y0, "u": None}

            def emit_layer(st, i):
                y0, y_in = st["y0"], st["y_in"]
                if i == 0:
                    st["u"] = upool.tile([128, NM, BT], f32, tag="u",
                                         name="u")
                u = st["u"]
                # --- S1 V-stage ---
                v_ps = [ps_vw.tile([128, BT], f32, tag="vw",
                                   name=f"v{pr_}") for pr_ in range(2)]
                for pr in range(2):
                    for k in range(KC):
                        nc.tensor.matmul(
                            v_ps[pr][:],
                            vp_sb[i][:, (k * 2 + pr) * 128:
                                     (k * 2 + pr + 1) * 128],
                            y_in[:, k, :],
                            start=(k == 0), stop=(k == KC - 1))
                # --- S0 gating logits ---
                glog = ps_g.tile([E, BT], f32, tag="g")
                for k in range(KC):
                    nc.tensor.matmul(glog[:], gt_sb[:, k * E:(k + 1) * E],
                                     y_in[:, k, :],
                                     start=(k == 0), stop=(k == KC - 1))
                t_sb = [tp.tile([128, BT], f16, tag="t",
                                name=f"t{pr_}") for pr_ in range(2)]
                for pr in range(2):
                    nc.scalar.activation(t_sb[pr][:], v_ps[pr][:], ACTF.Tanh)
                eg = gp.tile([E, BT], f32, tag="eg")
                nc.scalar.activation(eg[:], glog[:], ACTF.Exp)
                z4 = ps_g.tile([E, BT], f32, tag="g")
                nc.tensor.matmul(z4[:], ones_sb[:], eg[:], start=True,
                                 stop=True)
                rz4 = gp.tile([E, BT], f32, tag="rz", bufs=1)
                nc.vector.reciprocal_approx_fast(out=rz4[:], in_=z4[:])
                gn4 = gp.tile([E, BT], f32, tag="gn")
                nc.vector.tensor_mul(out=gn4[:], in0=eg[:], in1=rz4[:])
                # --- S2 C-stage ---
                w_ps = [ps_vw.tile([128, BT], f32, tag="vw",
                                   name=f"w{pr_}") for pr_ in range(2)]
                for pr in range(2):
                    nc.tensor.matmul(w_ps[pr][:],
                                     cb_sb[i][:, pr * 128:(pr + 1) * 128],
                                     t_sb[pr][:], start=True, stop=True)
                gbc_ps = [ps_gbc.tile([128, BT], f32, tag="gbc",
                                      name=f"gbc{pr_}") for pr_ in range(2)]
                for pr in range(2):
                    nc.tensor.matmul(gbc_ps[pr][:], sel_sb[:, pr, :],
                                     gn4[:], start=True, stop=True)
                wg_sb = []
                for pr in range(2):
                    tw = twp.tile([128, BT], f32, tag="tw")
                    nc.scalar.activation(tw[:], w_ps[pr][:], ACTF.Tanh)
                    wg = wgp.tile([128, BT], f16, tag="wg")
                    nc.vector.tensor_mul(out=wg[:], in0=tw[:],
                                         in1=gbc_ps[pr][:])
                    wg_sb.append(wg)
                # --- S3 U-stage + u update + y materialization ---
                last = (i == L - 1)
                y_out = yp.tile([128, KC, BT], f16, tag="y")
                for m in range(NM):
                    acc = ps_acc.tile([128, BT], f32, tag="acc")
                    nc.tensor.matmul(acc[:],
                                     up_sb[i][:, m * 256:m * 256 + 128],
                                     wg_sb[0][:], start=True, stop=False)
                    nc.tensor.matmul(acc[:],
                                     up_sb[i][:, m * 256 + 128:m * 256 + 256],
                                     wg_sb[1][:], start=False, stop=True)
                    bcol = bia_sb[:, i * NM + m: i * NM + m + 1]
                    if i == 0:
                        # u = acc + (1 + bias_0)  (the 1+ is folded into BIA)
                        nc.scalar.activation(u[:, m, :], acc[:],
                                             ACTF.Identity, bias=bcol)
                    else:
                        nc.vector.scalar_tensor_tensor(
                            out=u[:, m, :], in0=acc[:], scalar=bcol,
                            in1=u[:, m, :], op0=ALU.add, op1=ALU.add)
                    nc.vector.tensor_mul(out=y_out[:, m, :], in0=y0[:, m, :],
                                         in1=u[:, m, :])
                st["y_in"] = y_out

            def store_chunk(st, cidx):
                """Transpose y back to [BT, D] rows and DMA out."""
                r0 = cidx * BT
                y = st["y_in"]
                for bi in range(4):
                    po = ps_tr.tile([128, D], f16, tag="tr", name="po")
                    for dj in range(KC):
                        nc.tensor.transpose(
                            po[:, dj * 128:(dj + 1) * 128],
                            y[:, dj, bi * 128:(bi + 1) * 128], id_sb[:])
                    ot = orp.tile([128, D], f16, tag="or")
                    nc.scalar.activation(ot[:], po[:], ACTF.Copy)
                    nc.sync.dma_start(
                        out=OUT[r0 + bi * 128:r0 + (bi + 1) * 128, :],
                        in_=ot[:])

            for cidx in range(NCHUNK):
                st = load_chunk(cidx)
                for i in range(L):
                    emit_layer(st, i)
                store_chunk(st, cidx)
    nc.compile()
    return nc


def _get_nc():
    if "nc" not in _CACHE:
        _CACHE["nc"] = _build()
    return _CACHE["nc"]


def _prep_inputs(inputs, U, V, C, G, bias):
    """Host-side prep: fp16 x (row layout) + one packed fp16 weight blob."""
    f16 = np.float16
    x16 = inputs.astype(f16)

    W2 = np.zeros((128, WCOLS), f16)
    # gating [128, KC*E]: G.T [D, E] -> [KC, 128, E] -> [128, KC, E]
    W2[:, OFF_GT:OFF_GT + KC * E] = (
        G.T.reshape(KC, 128, E).transpose(1, 0, 2).reshape(128, KC * E))
    # bias [128, L*NM] with the residual "1 +" folded into layer 0
    biasm = bias.astype(np.float32, copy=True)
    biasm[0] += 1.0
    W2[:, OFF_BIA:OFF_BIA + L * NM] = (
        biasm.reshape(L, NM, 128).transpose(2, 0, 1).reshape(128, L * NM))
    # V packed pairs: [L, KC, 128, 2, 128] -> per layer [128, KC*2*128]
    VPh = V.transpose(0, 2, 1, 3).reshape(L, D, E * R).reshape(
        L, KC, 128, 2, 128)
    for i in range(L):
        W2[:, OFF_VP + i * KC * 256:OFF_VP + (i + 1) * KC * 256] = (
            VPh[i].transpose(1, 0, 2, 3).reshape(128, KC * 256))
    # C block-diagonal transposed: [L, 2, 128, 128] -> [128, 2*128]
    CBh = np.zeros((L, 2, 128, 128), np.float32)
    for i in range(L):
        for pr in range(2):
            CBh[i, pr, :64, :64] = C[i, 2 * pr].T
            CBh[i, pr, 64:, 64:] = C[i, 2 * pr + 1].T
    for i in range(L):
        W2[:, OFF_CB + i * 256:OFF_CB + (i + 1) * 256] = (
            CBh[i].transpose(1, 0, 2).reshape(128, 256))
    # U packed: [L, 2, 128, NM, 128] -> per layer [128, 2*NM*128] with the
    # m-major order the kernel indexes: [:, m*256 + pr*128 + col]
    UPh = U.transpose(0, 1, 3, 2).reshape(L, E * R, D).reshape(
        L, 2, 128, NM, 128)
    for i in range(L):
        W2[:, OFF_UP + i * NM * 256:OFF_UP + (i + 1) * NM * 256] = (
            UPh[i].transpose(1, 2, 0, 3).reshape(128, NM * 256))

    in_maps = []
    for c in range(NCORES):
        in_maps.append({
            "X": x16[c * BC:(c + 1) * BC],
            "WSH": W2[c * WROWS_SH:(c + 1) * WROWS_SH],
        })
    return in_maps


def kernel(inputs, U, V, C, G, bias):
    from concourse.bass_utils import run_bass_kernel_spmd

    inputs = np.asarray(inputs, dtype=np.float32)
    U = np.asarray(U, dtype=np.float32)
    V = np.asarray(V, dtype=np.float32)
    C = np.asarray(C, dtype=np.float32)
    G = np.asarray(G, dtype=np.float32)
    bias = np.asarray(bias, dtype=np.float32)

    nc = _get_nc()
    in_maps = _prep_inputs(inputs, U, V, C, G, bias)
    res = run_bass_kernel_spmd(nc, in_maps, core_ids=list(range(NCORES)))
    out = np.concatenate([res.results[c]["OUT"] for c in range(NCORES)],
                         axis=0)
    return out.astype(np.float32)


# revision 10
# speedup vs baseline: 3.1337x; 1.4128x over previous
"""CrossNetMix (moe_routing) Trainium2 Bass kernel — transfer-optimized.

Math (per layer i, softmax gates g sum to 1 over E):
    x_{l+1} = x_l + x0 * (sum_e g_e * U_e @ tanh(C_e @ tanh(V_e^T x_l)) + bias_i)

The residual chain collapses to x_L = x0 * (1 + sum_i (acc_i + bias_i)), so we
carry u_i = 1 + sum_{j<i} (acc_j + bias_j) and materialize y_i = y0 * u_i
(transposed space y = x^T) only as matmul input.

The axon host<->device tunnel runs at ~40 MB/s full-duplex, so wall time is
dominated by transfer bytes and overlap, not compute (~0.2 ms of HW work):
  - x moves as fp16 in natural [B, D] row layout (no host transpose);
    transposed on device via PE-identity matmuls; output likewise returns
    as fp16 [B, D] rows.
  - all weights are packed host-side into one fp16 blob [128, 13112],
    sharded 8-ways by rows (each core uploads 1/8 = 0.42 MB) and
    reassembled on device with a NeuronLink AllGather.
  - constant matrices (identity / expert-select / ones) ride inside the
    NEFF via inline_tensor — zero upload.
  - the batch is processed as 4 pipeline slices (4096 rows each): slice
    k's download overlaps slice k+1's upload (full-duplex tunnel), the
    donated output buffers are created on device (no zeros upload), and
    the weight blob is uploaded once per call and reused device-resident.
Gating softmax stays in fp32 (fp16 exp would overflow for |logit| > 11);
the u accumulator stays fp32; everything else computes from fp16 operands
with fp32 PSUM accumulation (~1e-3 matmul accuracy, vs the 2e-2 gate).
If the fast runner hits any problem, kernel() falls back to a plain
full-size run_bass_kernel_spmd call.
"""

import numpy as np

B, D, R, E, L = 16384, 1024, 64, 4, 3
NCORES = 8
BC = B // NCORES            # batch rows per core
BT = 512                    # batch columns per chunk (fp32 PSUM bank capacity)
KC = D // 128               # K-chunks over D
NM = D // 128               # M-chunks over D

NSLICE = 4                  # pipeline slices per kernel() call
BCS = BC // NSLICE          # rows per core per slice
ROWS = NCORES * BCS         # global rows per slice

# packed fp16 weight blob [128, WCOLS] column offsets
OFF_GT = 0                  # [128, KC*E]        gating weights
OFF_BIA = OFF_GT + KC * E   # [128, L*NM]        bias (+1 folded into layer 0)
OFF_VP = OFF_BIA + L * NM   # L x [128, KC*2*128]
OFF_CB = OFF_VP + L * KC * 2 * 128   # L x [128, 2*128]
OFF_UP = OFF_CB + L * 2 * 128        # L x [128, 2*NM*128]
WCOLS = OFF_UP + L * 2 * NM * 128
WROWS_SH = 128 // NCORES    # blob rows uploaded per core

_CACHE = {}


def _build(bcs):
    import concourse.mybir as mybir
    import concourse.bacc as bacc
    import concourse.tile as tile

    nchunk = bcs // BT
    f32 = mybir.dt.float32
    f16 = mybir.dt.float16
    ALU = mybir.AluOpType
    ACTF = mybir.ActivationFunctionType

    nc = bacc.Bacc("TRN2", target_bir_lowering=False, debug=False,
                   num_devices=NCORES)

    X = nc.dram_tensor("X", [bcs, D], f16, kind="ExternalInput")
    WSH = nc.dram_tensor("WSH", [WROWS_SH, WCOLS], f16, kind="ExternalInput")
    OUT = nc.dram_tensor("OUT", [bcs, D], f16, kind="ExternalOutput")

    wbnc = nc.dram_tensor("wbnc", [WROWS_SH, WCOLS], f16)
    wall = nc.dram_tensor("wall", [128, WCOLS], f16, addr_space="Shared")

    # inline constants (shipped inside the NEFF, no upload)
    ident_h = np.eye(128, dtype=np.float16)
    sel_h = np.zeros((E, 2, 128), np.float32)
    for e in range(E):
        sel_h.reshape(E, 256)[e, e * 64:(e + 1) * 64] = 1.0
    ones_h = np.ones((E, E), np.float32)

    with tile.TileContext(nc) as tc:
        with (
            tc.tile_pool(name="wts", bufs=1) as wts,
            tc.tile_pool(name="xrp", bufs=2) as xrp,
            tc.tile_pool(name="y0p", bufs=3) as y0p,
            tc.tile_pool(name="yp", bufs=3) as yp,
            tc.tile_pool(name="up", bufs=2) as upool,
            tc.tile_pool(name="tp", bufs=2) as tp,
            tc.tile_pool(name="twp", bufs=2) as twp,
            tc.tile_pool(name="wgp", bufs=2) as wgp,
            tc.tile_pool(name="gp", bufs=2) as gp,
            tc.tile_pool(name="orp", bufs=2) as orp,
            tc.tile_pool(name="ps_g", bufs=1, space="PSUM") as ps_g,
            tc.tile_pool(name="ps_gbc", bufs=2, space="PSUM") as ps_gbc,
            tc.tile_pool(name="ps_vw", bufs=2, space="PSUM") as ps_vw,
            tc.tile_pool(name="ps_acc", bufs=2, space="PSUM") as ps_acc,
            tc.tile_pool(name="ps_tr", bufs=1, space="PSUM") as ps_tr,
        ):
            # gather the 8 weight shards over NeuronLink ASAP
            nc.sync.dma_start(out=wbnc[:, :], in_=WSH[:, :])
            nc.gpsimd.collective_compute(
                "AllGather", mybir.AluOpType.bypass,
                replica_groups=[list(range(NCORES))],
                ins=[wbnc.ap()], outs=[wall.ap()])

            id_sb = wts.tile([128, 128], f16, tag="id")
            nc.sync.dma_start(out=id_sb[:], in_=nc.inline_tensor(
                ident_h, name="ident")[:, :])
            sel_sb = wts.tile([E, 2, 128], f32, tag="sel")
            nc.sync.dma_start(out=sel_sb[:], in_=nc.inline_tensor(
                sel_h, name="sel")[:, :, :])
            ones_sb = wts.tile([E, E], f32, tag="ones")
            nc.sync.dma_start(out=ones_sb[:], in_=nc.inline_tensor(
                ones_h, name="ones")[:, :])

            # weight tiles from the gathered blob
            gt_sb = wts.tile([128, KC * E], f16, tag="gt")
            nc.sync.dma_start(out=gt_sb[:],
                              in_=wall[:, OFF_GT:OFF_GT + KC * E])
            bia16 = wts.tile([128, L * NM], f16, tag="bia16")
            nc.sync.dma_start(out=bia16[:],
                              in_=wall[:, OFF_BIA:OFF_BIA + L * NM])
            bia_sb = wts.tile([128, L * NM], f32, tag="bia")
            nc.scalar.activation(bia_sb[:], bia16[:], ACTF.Copy)
            vp_sb, cb_sb, up_sb = [], [], []
            for i in range(L):
                vp = wts.tile([128, KC * 2 * 128], f16, tag=f"vp{i}")
                nc.sync.dma_start(
                    out=vp[:], in_=wall[:, OFF_VP + i * KC * 256:
                                        OFF_VP + (i + 1) * KC * 256])
                vp_sb.append(vp)
                cb = wts.tile([128, 2 * 128], f16, tag=f"cb{i}")
                nc.sync.dma_start(
                    out=cb[:], in_=wall[:, OFF_CB + i * 256:
                                        OFF_CB + (i + 1) * 256])
                cb_sb.append(cb)
                up = wts.tile([128, 2 * NM * 128], f16, tag=f"up{i}")
                nc.sync.dma_start(
                    out=up[:], in_=wall[:, OFF_UP + i * NM * 256:
                                        OFF_UP + (i + 1) * NM * 256])
                up_sb.append(up)

            def load_chunk(cidx):
                """DMA 512 batch rows and transpose to y0 [128, KC, BT]."""
                r0 = cidx * BT
                xr = []
                for bi in range(4):
                    t_ = xrp.tile([128, D], f16, tag=f"xr{bi}")
                    nc.sync.dma_start(
                        out=t_[:], in_=X[r0 + bi * 128:r0 + (bi + 1) * 128, :])
                    xr.append(t_)
                y0 = y0p.tile([128, KC, BT], f16, tag="y0",
                              name=f"y0_{cidx}")
                for dj in range(KC):
                    pt = ps_tr.tile([128, D], f16, tag="tr", name="pt")
                    for bi in range(4):
                        nc.tensor.transpose(
                            pt[:, bi * 128:(bi + 1) * 128],
                            xr[bi][:, dj * 128:(dj + 1) * 128], id_sb[:])
                    nc.scalar.activation(y0[:, dj, :], pt[:, :BT], ACTF.Copy)
                return {"y0": y0, "y_in": y0, "u": None}

            def emit_layer(st, i):
                y0, y_in = st["y0"], st["y_in"]
                if i == 0:
                    st["u"] = upool.tile([128, NM, BT], f32, tag="u",
                                         name="u")
                u = st["u"]
                # --- S1 V-stage ---
                v_ps = [ps_vw.tile([128, BT], f32, tag="vw",
                                   name=f"v{pr_}") for pr_ in range(2)]
                for pr in range(2):
                    for k in range(KC):
                        nc.tensor.matmul(
                            v_ps[pr][:],
                            vp_sb[i][:, (k * 2 + pr) * 128:
                                     (k * 2 + pr + 1) * 128],
                            y_in[:, k, :],
                            start=(k == 0), stop=(k == KC - 1))
                # --- S0 gating logits ---
                glog = ps_g.tile([E, BT], f32, tag="g")
                for k in range(KC):
                    nc.tensor.matmul(glog[:], gt_sb[:, k * E:(k + 1) * E],
                                     y_in[:, k, :],
                                     start=(k == 0), stop=(k == KC - 1))
                t_sb = [tp.tile([128, BT], f16, tag="t",
                                name=f"t{pr_}") for pr_ in range(2)]
                for pr in range(2):
                    nc.scalar.activation(t_sb[pr][:], v_ps[pr][:], ACTF.Tanh)
                eg = gp.tile([E, BT], f32, tag="eg")
                nc.scalar.activation(eg[:], glog[:], ACTF.Exp)
                z4 = ps_g.tile([E, BT], f32, tag="g")
                nc.tensor.matmul(z4[:], ones_sb[:], eg[:], start=True,
                                 stop=True)
                rz4 = gp.tile([E, BT], f32, tag="rz", bufs=1)
                nc.vector.reciprocal_approx_fast(out=rz4[:], in_=z4[:])
                gn4 = gp.tile([E, BT], f32, tag="gn")
                nc.vector.tensor_mul(out=gn4[:], in0=eg[:], in1=rz4[:])
                # --- S2 C-stage ---
                w_ps = [ps_vw.tile([128, BT], f32, tag="vw",
                                   name=f"w{pr_}") for pr_ in range(2)]
                for pr in range(2):
                    nc.tensor.matmul(w_ps[pr][:],
                                     cb_sb[i][:, pr * 128:(pr + 1) * 128],
                                     t_sb[pr][:], start=True, stop=True)
                gbc_ps = [ps_gbc.tile([128, BT], f32, tag="gbc",
                                      name=f"gbc{pr_}") for pr_ in range(2)]
                for pr in range(2):
                    nc.tensor.matmul(gbc_ps[pr][:], sel_sb[:, pr, :],
                                     gn4[:], start=True, stop=True)
                wg_sb = []
                for pr in range(2):
                    tw = twp.tile([128, BT], f32, tag="tw")
                    nc.scalar.activation(tw[:], w_ps[pr][:], ACTF.Tanh)
                    wg = wgp.tile([128, BT], f16, tag="wg")
                    nc.vector.tensor_mul(out=wg[:], in0=tw[:],
                                         in1=gbc_ps[pr][:])
                    wg_sb.append(wg)
                # --- S3 U-stage + u update + y materialization ---
                y_out = yp.tile([128, KC, BT], f16, tag="y")
                for m in range(NM):
                    acc = ps_acc.tile([128, BT], f32, tag="acc")
                    nc.tensor.matmul(acc[:],
                                     up_sb[i][:, m * 256:m * 256 + 128],
                                     wg_sb[0][:], start=True, stop=False)
                    nc.tensor.matmul(acc[:],
                                     up_sb[i][:, m * 256 + 128:m * 256 + 256],
                                     wg_sb[1][:], start=False, stop=True)
                    bcol = bia_sb[:, i * NM + m: i * NM + m + 1]
                    if i == 0:
                        # u = acc + (1 + bias_0)  (the 1+ is folded into BIA)
                        nc.scalar.activation(u[:, m, :], acc[:],
                                             ACTF.Identity, bias=bcol)
                    else:
                        nc.vector.scalar_tensor_tensor(
                            out=u[:, m, :], in0=acc[:], scalar=bcol,
                            in1=u[:, m, :], op0=ALU.add, op1=ALU.add)
                    nc.vector.tensor_mul(out=y_out[:, m, :], in0=y0[:, m, :],
                                         in1=u[:, m, :])
                st["y_in"] = y_out

            def store_chunk(st, cidx):
                """Transpose y back to [BT, D] rows and DMA out."""
                r0 = cidx * BT
                y = st["y_in"]
                for bi in range(4):
                    po = ps_tr.tile([128, D], f16, tag="tr", name="po")
                    for dj in range(KC):
                        nc.tensor.transpose(
                            po[:, dj * 128:(dj + 1) * 128],
                            y[:, dj, bi * 128:(bi + 1) * 128], id_sb[:])
                    ot = orp.tile([128, D], f16, tag="or")
                    nc.scalar.activation(ot[:], po[:], ACTF.Copy)
                    nc.sync.dma_start(
                        out=OUT[r0 + bi * 128:r0 + (bi + 1) * 128, :],
                        in_=ot[:])

            for cidx in range(nchunk):
                st = load_chunk(cidx)
                for i in range(L):
                    emit_layer(st, i)
                store_chunk(st, cidx)
    nc.compile()
    return nc


def _get_nc(bcs):
    key = ("nc", bcs)
    if key not in _CACHE:
        _CACHE[key] = _build(bcs)
    return _CACHE[key]


def _pack_weights(U, V, C, G, bias):
    """One fp16 blob [128, WCOLS] holding every weight in its SBUF layout."""
    W2 = np.zeros((128, WCOLS), np.float16)
    # gating [128, KC*E]: G.T [D, E] -> [KC, 128, E] -> [128, KC, E]
    W2[:, OFF_GT:OFF_GT + KC * E] = (
        G.T.reshape(KC, 128, E).transpose(1, 0, 2).reshape(128, KC * E))
    # bias [128, L*NM] with the residual "1 +" folded into layer 0
    biasm = bias.astype(np.float32, copy=True)
    biasm[0] += 1.0
    W2[:, OFF_BIA:OFF_BIA + L * NM] = (
        biasm.reshape(L, NM, 128).transpose(2, 0, 1).reshape(128, L * NM))
    # V packed pairs: [L, KC, 128, 2, 128] -> per layer [128, KC*2*128]
    VPh = V.transpose(0, 2, 1, 3).reshape(L, D, E * R).reshape(
        L, KC, 128, 2, 128)
    for i in range(L):
        W2[:, OFF_VP + i * KC * 256:OFF_VP + (i + 1) * KC * 256] = (
            VPh[i].transpose(1, 0, 2, 3).reshape(128, KC * 256))
    # C block-diagonal transposed: [L, 2, 128, 128] -> [128, 2*128]
    CBh = np.zeros((L, 2, 128, 128), np.float32)
    for i in range(L):
        for pr in range(2):
            CBh[i, pr, :64, :64] = C[i, 2 * pr].T
            CBh[i, pr, 64:, 64:] = C[i, 2 * pr + 1].T
    for i in range(L):
        W2[:, OFF_CB + i * 256:OFF_CB + (i + 1) * 256] = (
            CBh[i].transpose(1, 0, 2).reshape(128, 256))
    # U packed: [L, 2, 128, NM, 128] -> per layer [128, NM, 2, 128] flat,
    # matching the kernel's [:, m*256 + pr*128 + col] indexing
    UPh = U.transpose(0, 1, 3, 2).reshape(L, E * R, D).reshape(
        L, 2, 128, NM, 128)
    for i in range(L):
        W2[:, OFF_UP + i * NM * 256:OFF_UP + (i + 1) * NM * 256] = (
            UPh[i].transpose(1, 2, 0, 3).reshape(128, NM * 256))
    return W2


class _FastRunner:
    """Pipelined multi-slice executor built on the bass_exec PJRT primitive.

    Mirrors run_bass_via_pjrt's binding protocol exactly, but keeps the
    jitted executable cached, keeps the weight blob device-resident across
    slices, creates the donated output buffers on device, and overlaps
    slice k's download with slice k+1's upload (the tunnel is full-duplex).
    """

    def __init__(self):
        import concurrent.futures as cf
        import jax
        import concourse.mybir as mybir
        from jax.experimental.shard_map import shard_map
        from jax.sharding import Mesh, PartitionSpec, NamedSharding
        from concourse import bass2jax

        nc = _get_nc(BCS)
        if nc.dbg_addr is not None or nc.dbg_callbacks:
            raise RuntimeError("fast runner needs debug=False")
        bass2jax.install_neuronx_cc_hook()
        self._jax = jax

        partition_name = (nc.partition_id_tensor.name
                          if nc.partition_id_tensor else None)
        in_names, out_names, out_avals, zero_shapes = [], [], [], []
        for alloc in nc.m.functions[0].allocations:
            if not isinstance(alloc, mybir.MemoryLocationSet):
                continue
            name = alloc.memorylocations[0].name
            if alloc.kind == "ExternalInput":
                if name != partition_name:
                    in_names.append(name)
            elif alloc.kind == "ExternalOutput":
                shape = tuple(alloc.tensor_shape)
                dtype = mybir.dt.np(alloc.dtype)
                out_names.append(name)
                out_avals.append(jax.core.ShapedArray(shape, dtype))
                zero_shapes.append((shape, dtype))
        assert set(in_names) == {"X", "WSH"} and out_names == ["OUT"], (
            in_names, out_names)
        n_params = len(in_names)
        self._arg_names = list(in_names)
        all_names = in_names + out_names
        if partition_name is not None:
            all_names = all_names + [partition_name]

        def _body(*args):
            operands = list(args)
            if partition_name is not None:
                operands.append(bass2jax.partition_id_tensor())
            outs = bass2jax._bass_exec_p.bind(
                *operands,
                out_avals=tuple(out_avals),
                in_names=tuple(all_names),
                out_names=tuple(out_names),
                lowering_input_output_aliases=(),
                sim_require_finite=True,
                sim_require_nnan=True,
                nc=nc,
            )
            return tuple(outs)

        devices = jax.devices()[:NCORES]
        assert len(devices) == NCORES, devices
        mesh = Mesh(np.asarray(devices), ("core",))
        self._sharding = NamedSharding(mesh, PartitionSpec("core"))
        n_all = n_params + len(out_names)
        in_specs = (PartitionSpec("core"),) * n_all
        out_specs = (PartitionSpec("core"),) * len(out_names)
        donate = tuple(range(n_params, n_all))
        self._jitted = jax.jit(
            shard_map(_body, mesh=mesh, in_specs=in_specs,
                      out_specs=out_specs, check_rep=False),
            donate_argnums=donate, keep_unused=True)
        gshape, gdtype = zero_shapes[0]
        self._zjit = jax.jit(
            lambda: jax.numpy.zeros((NCORES * gshape[0],) + gshape[1:],
                                    gdtype),
            out_shardings=self._sharding)
        self._pool = cf.ThreadPoolExecutor(max_workers=1)

    def run(self, x16, W2):
        jax = self._jax
        wdev = jax.device_put(W2, self._sharding)
        ops = {"WSH": wdev}
        futs = []
        for k in range(NSLICE):
            ops["X"] = jax.device_put(x16[k * ROWS:(k + 1) * ROWS],
                                      self._sharding)
            args = tuple(ops[n] for n in self._arg_names)
            out, = self._jitted(*args, self._zjit())
            futs.append(self._pool.submit(np.asarray, out))
        parts = [f.result() for f in futs]
        return np.concatenate(parts, axis=0)


def _get_fast():
    if "fast" not in _CACHE:
        _CACHE["fast"] = _FastRunner()
    return _CACHE["fast"]


def _run_fallback(x16, W2):
    """Plain single-shot path via run_bass_kernel_spmd (full batch)."""
    from concourse.bass_utils import run_bass_kernel_spmd
    nc = _get_nc(BC)
    in_maps = []
    for c in range(NCORES):
        in_maps.append({
            "X": x16[c * BC:(c + 1) * BC],
            "WSH": W2[c * WROWS_SH:(c + 1) * WROWS_SH],
        })
    res = run_bass_kernel_spmd(nc, in_maps, core_ids=list(range(NCORES)))
    return np.concatenate([res.results[c]["OUT"] for c in range(NCORES)],
                          axis=0)


def kernel(inputs, U, V, C, G, bias):
    inputs = np.asarray(inputs, dtype=np.float32)
    U = np.asarray(U, dtype=np.float32)
    V = np.asarray(V, dtype=np.float32)
    C = np.asarray(C, dtype=np.float32)
    G = np.asarray(G, dtype=np.float32)
    bias = np.asarray(bias, dtype=np.float32)

    x16 = inputs.astype(np.float16)
    W2 = _pack_weights(U, V, C, G, bias)
    try:
        out16 = _get_fast().run(x16, W2)
    except Exception:
        import sys, traceback
        traceback.print_exc()
        print("kernel: fast path failed; using fallback", file=sys.stderr)
        out16 = _run_fallback(x16, W2)
    return out16.astype(np.float32)


# revision 14
# speedup vs baseline: 4.5706x; 1.4585x over previous
"""CrossNetMix (moe_routing) Trainium2 Bass kernel — transfer-optimized.

Math (per layer i, softmax gates g sum to 1 over E):
    x_{l+1} = x_l + x0 * (sum_e g_e * U_e @ tanh(C_e @ tanh(V_e^T x_l)) + bias_i)

The residual chain collapses to x_L = x0 * u with
u = 1 + sum_i (gated_moe_i + bias_i), so the kernel carries u and
materializes y_i = y0 * u_i (transposed space y = x^T) as matmul input.

The axon host<->device tunnel moves ~46 MB/s TOTAL (shared across both
directions), so wall time ~= bytes moved / 46MB/s; on-device compute is
~0.2 ms.  Transfer plan (~36 MB per call vs 250 MB for the naive port):
  - x uploads as int8 with a per-row scale (16 MB + 32 KB): dequantized to
    fp16 on device, then PE-transposed into column-major y0.
  - the device returns U = u (not y = x0*u) quantized int8 with per-row
    scales computed on device; the host computes x0_f32 * (uq * su), so
    the final product uses EXACT x0 — x-quantization only enters through
    the layer nonlinearities (attenuated), keeping l2 error ~8e-3 vs the
    2e-2 gate.
  - all weights pack into one fp16 blob [128, 13112], sharded 8-ways
    (0.42 MB uploaded per core) and reassembled on device with a
    NeuronLink AllGather; constants (identity/select/ones) ship inside
    the NEFF via inline_tensor.
  - the batch runs as 4 pipeline slices whose downloads overlap later
    uploads; donated output buffers are created on device (no zeros
    upload); the weight blob is uploaded once per call and reused
    device-resident across slices.
Gating softmax stays in fp32 (fp16 exp would overflow for |logit| > 11);
the u accumulator stays fp32; matmuls take fp16 operands with fp32 PSUM
accumulation.  If the fast runner hits any problem, kernel() falls back
to a plain run_bass_kernel_spmd call on the same NEFF.
"""

import numpy as np

B, D, R, E, L = 16384, 1024, 64, 4, 3
NCORES = 8
BC = B // NCORES            # batch rows per core
BT = 512                    # batch columns per chunk (fp32 PSUM bank capacity)
KC = D // 128               # K-chunks over D
NM = D // 128               # M-chunks over D

NSLICE = 4                  # pipeline slices per kernel() call
BCS = BC // NSLICE          # rows per core per slice
ROWS = NCORES * BCS         # global rows per slice

QCLIP = 126.0               # int8 headroom so approx-reciprocal can't wrap

# packed fp16 weight blob [128, WCOLS] column offsets
OFF_GT = 0                  # [128, KC*E]        gating weights
OFF_BIA = OFF_GT + KC * E   # [128, L*NM]        bias (+1 folded into layer 0)
OFF_VP = OFF_BIA + L * NM   # L x [128, KC*2*128]
OFF_CB = OFF_VP + L * KC * 2 * 128   # L x [128, 2*128]
OFF_UP = OFF_CB + L * 2 * 128        # L x [128, 2*NM*128]
WCOLS = OFF_UP + L * 2 * NM * 128
WROWS_SH = 128 // NCORES    # blob rows uploaded per core

_CACHE = {}


def _build(bcs):
    import concourse.mybir as mybir
    import concourse.bacc as bacc
    import concourse.tile as tile

    nchunk = bcs // BT
    f32 = mybir.dt.float32
    f16 = mybir.dt.float16
    i8 = mybir.dt.int8
    ALU = mybir.AluOpType
    ACTF = mybir.ActivationFunctionType

    nc = bacc.Bacc("TRN2", target_bir_lowering=False, debug=False,
                   num_devices=NCORES)

    XQ = nc.dram_tensor("XQ", [bcs, D], i8, kind="ExternalInput")
    SX = nc.dram_tensor("SX", [bcs, 1], f16, kind="ExternalInput")
    WSH = nc.dram_tensor("WSH", [WROWS_SH, WCOLS], f16, kind="ExternalInput")
    UQ = nc.dram_tensor("UQ", [bcs, D], i8, kind="ExternalOutput")
    US = nc.dram_tensor("US", [bcs, 1], f16, kind="ExternalOutput")

    wbnc = nc.dram_tensor("wbnc", [WROWS_SH, WCOLS], f16)
    wall = nc.dram_tensor("wall", [128, WCOLS], f16, addr_space="Shared")

    # inline constants (shipped inside the NEFF, no upload)
    ident_h = np.eye(128, dtype=np.float16)
    sel_h = np.zeros((E, 2, 128), np.float32)
    for e in range(E):
        sel_h.reshape(E, 256)[e, e * 64:(e + 1) * 64] = 1.0
    ones_h = np.ones((E, E), np.float32)

    with tile.TileContext(nc) as tc:
        with (
            tc.tile_pool(name="wts", bufs=1) as wts,
            tc.tile_pool(name="xqp", bufs=2) as xqp,
            tc.tile_pool(name="xrp", bufs=2) as xrp,
            tc.tile_pool(name="sxp", bufs=2) as sxp,
            tc.tile_pool(name="y0p", bufs=3) as y0p,
            tc.tile_pool(name="yp", bufs=3) as yp,
            tc.tile_pool(name="up", bufs=2) as upool,
            tc.tile_pool(name="tp", bufs=2) as tp,
            tc.tile_pool(name="twp", bufs=2) as twp,
            tc.tile_pool(name="wgp", bufs=2) as wgp,
            tc.tile_pool(name="gp", bufs=2) as gp,
            tc.tile_pool(name="orp", bufs=2) as orp,
            tc.tile_pool(name="qsp", bufs=2) as qsp,
            tc.tile_pool(name="ps_g", bufs=1, space="PSUM") as ps_g,
            tc.tile_pool(name="ps_gbc", bufs=2, space="PSUM") as ps_gbc,
            tc.tile_pool(name="ps_vw", bufs=2, space="PSUM") as ps_vw,
            tc.tile_pool(name="ps_acc", bufs=2, space="PSUM") as ps_acc,
            tc.tile_pool(name="ps_tr", bufs=1, space="PSUM") as ps_tr,
        ):
            # gather the 8 weight shards over NeuronLink ASAP
            nc.sync.dma_start(out=wbnc[:, :], in_=WSH[:, :])
            nc.gpsimd.collective_compute(
                "AllGather", mybir.AluOpType.bypass,
                replica_groups=[list(range(NCORES))],
                ins=[wbnc.ap()], outs=[wall.ap()])

            id_sb = wts.tile([128, 128], f16, tag="id")
            nc.sync.dma_start(out=id_sb[:], in_=nc.inline_tensor(
                ident_h, name="ident")[:, :])
            sel_sb = wts.tile([E, 2, 128], f32, tag="sel")
            nc.sync.dma_start(out=sel_sb[:], in_=nc.inline_tensor(
                sel_h, name="sel")[:, :, :])
            ones_sb = wts.tile([E, E], f32, tag="ones")
            nc.sync.dma_start(out=ones_sb[:], in_=nc.inline_tensor(
                ones_h, name="ones")[:, :])

            # weight tiles from the gathered blob
            gt_sb = wts.tile([128, KC * E], f16, tag="gt")
            nc.sync.dma_start(out=gt_sb[:],
                              in_=wall[:, OFF_GT:OFF_GT + KC * E])
            bia16 = wts.tile([128, L * NM], f16, tag="bia16")
            nc.sync.dma_start(out=bia16[:],
                              in_=wall[:, OFF_BIA:OFF_BIA + L * NM])
            bia_sb = wts.tile([128, L * NM], f32, tag="bia")
            nc.scalar.activation(bia_sb[:], bia16[:], ACTF.Copy)
            vp_sb, cb_sb, up_sb = [], [], []
            for i in range(L):
                vp = wts.tile([128, KC * 2 * 128], f16, tag=f"vp{i}")
                nc.sync.dma_start(
                    out=vp[:], in_=wall[:, OFF_VP + i * KC * 256:
                                        OFF_VP + (i + 1) * KC * 256])
                vp_sb.append(vp)
                cb = wts.tile([128, 2 * 128], f16, tag=f"cb{i}")
                nc.sync.dma_start(
                    out=cb[:], in_=wall[:, OFF_CB + i * 256:
                                        OFF_CB + (i + 1) * 256])
                cb_sb.append(cb)
                up = wts.tile([128, 2 * NM * 128], f16, tag=f"up{i}")
                nc.sync.dma_start(
                    out=up[:], in_=wall[:, OFF_UP + i * NM * 256:
                                        OFF_UP + (i + 1) * NM * 256])
                up_sb.append(up)

            def load_chunk(cidx):
                """DMA 512 int8 rows, dequantize, transpose to y0."""
                r0 = cidx * BT
                xr = []
                for bi in range(4):
                    rs = slice(r0 + bi * 128, r0 + (bi + 1) * 128)
                    xq = xqp.tile([128, D], i8, tag=f"xq{bi}")
                    nc.sync.dma_start(out=xq[:], in_=XQ[rs, :])
                    sx16 = sxp.tile([128, 1], f16, tag=f"sx{bi}")
                    nc.sync.dma_start(out=sx16[:], in_=SX[rs, :])
                    sxf = sxp.tile([128, 1], f32, tag=f"sxf{bi}")
                    nc.scalar.activation(sxf[:], sx16[:], ACTF.Copy)
                    t_ = xrp.tile([128, D], f16, tag=f"xr{bi}")
                    nc.scalar.activation(t_[:], xq[:], ACTF.Copy,
                                         scale=sxf[:, 0:1])
                    xr.append(t_)
                y0 = y0p.tile([128, KC, BT], f16, tag="y0",
                              name=f"y0_{cidx}")
                for dj in range(KC):
                    pt = ps_tr.tile([128, D], f16, tag="tr", name="pt")
                    for bi in range(4):
                        nc.tensor.transpose(
                            pt[:, bi * 128:(bi + 1) * 128],
                            xr[bi][:, dj * 128:(dj + 1) * 128], id_sb[:])
                    nc.scalar.activation(y0[:, dj, :], pt[:, :BT], ACTF.Copy)
                return {"y0": y0, "y_in": y0, "u": None}

            def emit_layer(st, i):
                y0, y_in = st["y0"], st["y_in"]
                if i == 0:
                    st["u"] = upool.tile([128, NM, BT], f32, tag="u",
                                         name="u")
                u = st["u"]
                # --- S1 V-stage ---
                v_ps = [ps_vw.tile([128, BT], f32, tag="vw",
                                   name=f"v{pr_}") for pr_ in range(2)]
                for pr in range(2):
                    for k in range(KC):
                        nc.tensor.matmul(
                            v_ps[pr][:],
                            vp_sb[i][:, (k * 2 + pr) * 128:
                                     (k * 2 + pr + 1) * 128],
                            y_in[:, k, :],
                            start=(k == 0), stop=(k == KC - 1))
                # --- S0 gating logits ---
                glog = ps_g.tile([E, BT], f32, tag="g")
                for k in range(KC):
                    nc.tensor.matmul(glog[:], gt_sb[:, k * E:(k + 1) * E],
                                     y_in[:, k, :],
                                     start=(k == 0), stop=(k == KC - 1))
                t_sb = [tp.tile([128, BT], f16, tag="t",
                                name=f"t{pr_}") for pr_ in range(2)]
                for pr in range(2):
                    nc.scalar.activation(t_sb[pr][:], v_ps[pr][:], ACTF.Tanh)
                eg = gp.tile([E, BT], f32, tag="eg")
                nc.scalar.activation(eg[:], glog[:], ACTF.Exp)
                z4 = ps_g.tile([E, BT], f32, tag="g")
                nc.tensor.matmul(z4[:], ones_sb[:], eg[:], start=True,
                                 stop=True)
                rz4 = gp.tile([E, BT], f32, tag="rz", bufs=1)
                nc.vector.reciprocal_approx_fast(out=rz4[:], in_=z4[:])
                gn4 = gp.tile([E, BT], f32, tag="gn")
                nc.vector.tensor_mul(out=gn4[:], in0=eg[:], in1=rz4[:])
                # --- S2 C-stage ---
                w_ps = [ps_vw.tile([128, BT], f32, tag="vw",
                                   name=f"w{pr_}") for pr_ in range(2)]
                for pr in range(2):
                    nc.tensor.matmul(w_ps[pr][:],
                                     cb_sb[i][:, pr * 128:(pr + 1) * 128],
                                     t_sb[pr][:], start=True, stop=True)
                gbc_ps = [ps_gbc.tile([128, BT], f32, tag="gbc",
                                      name=f"gbc{pr_}") for pr_ in range(2)]
                for pr in range(2):
                    nc.tensor.matmul(gbc_ps[pr][:], sel_sb[:, pr, :],
                                     gn4[:], start=True, stop=True)
                wg_sb = []
                for pr in range(2):
                    tw = twp.tile([128, BT], f32, tag="tw")
                    nc.scalar.activation(tw[:], w_ps[pr][:], ACTF.Tanh)
                    wg = wgp.tile([128, BT], f16, tag="wg")
                    nc.vector.tensor_mul(out=wg[:], in0=tw[:],
                                         in1=gbc_ps[pr][:])
                    wg_sb.append(wg)
                # --- S3 U-stage + u update + y materialization ---
                last = (i == L - 1)
                if not last:
                    y_out = yp.tile([128, KC, BT], f16, tag="y")
                for m in range(NM):
                    acc = ps_acc.tile([128, BT], f32, tag="acc")
                    nc.tensor.matmul(acc[:],
                                     up_sb[i][:, m * 256:m * 256 + 128],
                                     wg_sb[0][:], start=True, stop=False)
                    nc.tensor.matmul(acc[:],
                                     up_sb[i][:, m * 256 + 128:m * 256 + 256],
                                     wg_sb[1][:], start=False, stop=True)
                    bcol = bia_sb[:, i * NM + m: i * NM + m + 1]
                    if i == 0:
                        # u = acc + (1 + bias_0)  (the 1+ is folded into BIA)
                        nc.scalar.activation(u[:, m, :], acc[:],
                                             ACTF.Identity, bias=bcol)
                    else:
                        nc.vector.scalar_tensor_tensor(
                            out=u[:, m, :], in0=acc[:], scalar=bcol,
                            in1=u[:, m, :], op0=ALU.add, op1=ALU.add)
                    if not last:
                        nc.vector.tensor_mul(out=y_out[:, m, :],
                                             in0=y0[:, m, :], in1=u[:, m, :])
                if not last:
                    st["y_in"] = y_out

            def store_chunk(st, cidx):
                """Quantize u int8 per batch row and DMA out with scales."""
                r0 = cidx * BT
                u = st["u"]
                uh = yp.tile([128, KC, BT], f16, tag="y", name="uh")
                for m in range(NM):
                    nc.scalar.activation(uh[:, m, :], u[:, m, :], ACTF.Copy)
                for bi in range(4):
                    rs = slice(r0 + bi * 128, r0 + (bi + 1) * 128)
                    po = ps_tr.tile([128, D], f16, tag="tr", name="po")
                    for dj in range(KC):
                        nc.tensor.transpose(
                            po[:, dj * 128:(dj + 1) * 128],
                            uh[:, dj, bi * 128:(bi + 1) * 128], id_sb[:])
                    m1 = qsp.tile([128, 1], f32, tag="m1")
                    nc.vector.tensor_reduce(
                        out=m1[:], in_=po[:, :],
                        axis=mybir.AxisListType.X,
                        op=ALU.max, apply_absolute_value=True)
                    nc.vector.tensor_scalar_max(out=m1[:], in0=m1[:],
                                                scalar1=1e-6)
                    rq = qsp.tile([128, 1], f32, tag="rq")
                    nc.vector.reciprocal(out=rq[:], in_=m1[:])
                    nc.vector.tensor_scalar_mul(out=rq[:], in0=rq[:],
                                                scalar1=QCLIP)
                    qo = orp.tile([128, D], i8, tag="or")
                    nc.scalar.activation(qo[:], po[:], ACTF.Copy,
                                         scale=rq[:, 0:1])
                    us16 = qsp.tile([128, 1], f16, tag="us")
                    nc.scalar.activation(us16[:], m1[:], ACTF.Copy,
                                         scale=1.0 / QCLIP)
                    nc.sync.dma_start(out=UQ[rs, :], in_=qo[:])
                    nc.sync.dma_start(out=US[rs, :], in_=us16[:])

            for cidx in range(nchunk):
                st = load_chunk(cidx)
                for i in range(L):
                    emit_layer(st, i)
                store_chunk(st, cidx)
    nc.compile()
    return nc


def _get_nc(bcs):
    key = ("nc", bcs)
    if key not in _CACHE:
        _CACHE[key] = _build(bcs)
    return _CACHE[key]


def _pack_weights(U, V, C, G, bias):
    """One fp16 blob [128, WCOLS] holding every weight in its SBUF layout."""
    W2 = np.zeros((128, WCOLS), np.float16)
    # gating [128, KC*E]: G.T [D, E] -> [KC, 128, E] -> [128, KC, E]
    W2[:, OFF_GT:OFF_GT + KC * E] = (
        G.T.reshape(KC, 128, E).transpose(1, 0, 2).reshape(128, KC * E))
    # bias [128, L*NM] with the residual "1 +" folded into layer 0
    biasm = bias.astype(np.float32, copy=True)
    biasm[0] += 1.0
    W2[:, OFF_BIA:OFF_BIA + L * NM] = (
        biasm.reshape(L, NM, 128).transpose(2, 0, 1).reshape(128, L * NM))
    # V packed pairs: [L, KC, 128, 2, 128] -> per layer [128, KC*2*128]
    VPh = V.transpose(0, 2, 1, 3).reshape(L, D, E * R).reshape(
        L, KC, 128, 2, 128)
    for i in range(L):
        W2[:, OFF_VP + i * KC * 256:OFF_VP + (i + 1) * KC * 256] = (
            VPh[i].transpose(1, 0, 2, 3).reshape(128, KC * 256))
    # C block-diagonal transposed: [L, 2, 128, 128] -> [128, 2*128]
    CBh = np.zeros((L, 2, 128, 128), np.float32)
    for i in range(L):
        for pr in range(2):
            CBh[i, pr, :64, :64] = C[i, 2 * pr].T
            CBh[i, pr, 64:, 64:] = C[i, 2 * pr + 1].T
    for i in range(L):
        W2[:, OFF_CB + i * 256:OFF_CB + (i + 1) * 256] = (
            CBh[i].transpose(1, 0, 2).reshape(128, 256))
    # U packed: [L, 2, 128, NM, 128] -> per layer [128, NM, 2, 128] flat,
    # matching the kernel's [:, m*256 + pr*128 + col] indexing
    UPh = U.transpose(0, 1, 3, 2).reshape(L, E * R, D).reshape(
        L, 2, 128, NM, 128)
    for i in range(L):
        W2[:, OFF_UP + i * NM * 256:OFF_UP + (i + 1) * NM * 256] = (
            UPh[i].transpose(1, 2, 0, 3).reshape(128, NM * 256))
    return W2


def _quantize_x(x):
    """Per-row symmetric int8: returns (int8 [n,D], fp16 scales [n,1])."""
    sx = np.abs(x).max(axis=1)
    np.maximum(sx, 1e-20, out=sx)
    q = x * (QCLIP / sx)[:, None]
    np.rint(q, out=q)
    return q.astype(np.int8), (sx * (1.0 / QCLIP)).astype(np.float16)[:, None]


class _FastRunner:
    """Pipelined multi-slice executor built on the bass_exec PJRT primitive.

    Mirrors run_bass_via_pjrt's binding protocol exactly, but keeps the
    jitted executable cached, keeps the weight blob device-resident across
    slices, creates the donated output buffers on device, and overlaps
    slice k's download with slice k+1's upload (the tunnel is full-duplex).
    """

    def __init__(self):
        import concurrent.futures as cf
        import jax
        import concourse.mybir as mybir
        from jax.experimental.shard_map import shard_map
        from jax.sharding import Mesh, PartitionSpec, NamedSharding
        from concourse import bass2jax

        nc = _get_nc(BCS)
        if nc.dbg_addr is not None or nc.dbg_callbacks:
            raise RuntimeError("fast runner needs debug=False")
        bass2jax.install_neuronx_cc_hook()
        self._jax = jax

        partition_name = (nc.partition_id_tensor.name
                          if nc.partition_id_tensor else None)
        in_names, out_names, out_avals = [], [], []
        for alloc in nc.m.functions[0].allocations:
            if not isinstance(alloc, mybir.MemoryLocationSet):
                continue
            name = alloc.memorylocations[0].name
            if alloc.kind == "ExternalInput":
                if name != partition_name:
                    in_names.append(name)
            elif alloc.kind == "ExternalOutput":
                shape = tuple(alloc.tensor_shape)
                dtype = mybir.dt.np(alloc.dtype)
                out_names.append(name)
                out_avals.append(jax.core.ShapedArray(shape, dtype))
        assert set(in_names) == {"XQ", "SX", "WSH"}, in_names
        assert set(out_names) == {"UQ", "US"}, out_names
        n_params = len(in_names)
        self._arg_names = list(in_names)
        self._out_names = list(out_names)
        all_names = in_names + out_names
        if partition_name is not None:
            all_names = all_names + [partition_name]

        def _body(*args):
            operands = list(args)
            if partition_name is not None:
                operands.append(bass2jax.partition_id_tensor())
            outs = bass2jax._bass_exec_p.bind(
                *operands,
                out_avals=tuple(out_avals),
                in_names=tuple(all_names),
                out_names=tuple(out_names),
                lowering_input_output_aliases=(),
                sim_require_finite=True,
                sim_require_nnan=True,
                nc=nc,
            )
            return tuple(outs)

        devices = jax.devices()[:NCORES]
        assert len(devices) == NCORES, devices
        mesh = Mesh(np.asarray(devices), ("core",))
        self._sharding = NamedSharding(mesh, PartitionSpec("core"))
        n_all = n_params + len(out_names)
        in_specs = (PartitionSpec("core"),) * n_all
        out_specs = (PartitionSpec("core"),) * len(out_names)
        donate = tuple(range(n_params, n_all))
        self._jitted = jax.jit(
            shard_map(_body, mesh=mesh, in_specs=in_specs,
                      out_specs=out_specs, check_rep=False),
            donate_argnums=donate, keep_unused=True)
        zshapes = [((NCORES * a.shape[0],) + a.shape[1:], a.dtype)
                   for a in out_avals]
        self._zjit = jax.jit(
            lambda: tuple(jax.numpy.zeros(s, d) for s, d in zshapes),
            out_shardings=tuple(self._sharding for _ in zshapes))
        self._pool = cf.ThreadPoolExecutor(max_workers=2)

    def run(self, wdev, x0):
        """Returns the final f32 output [B, D] = x0 * dequant(u).

        Quantizes each slice on the host just before its upload, so host
        work overlaps earlier slices' transfers; likewise the dequant of
        slice k overlaps slice k+1's download.
        """
        jax = self._jax
        futs = []
        for k in range(NSLICE):
            rows = slice(k * ROWS, (k + 1) * ROWS)
            xq_k, sx_k = _quantize_x(x0[rows])
            ops = {"XQ": jax.device_put(xq_k, self._sharding),
                   "SX": jax.device_put(sx_k, self._sharding),
                   "WSH": wdev}
            args = tuple(ops[n] for n in self._arg_names)
            outs = self._jitted(*args, *self._zjit())
            byname = dict(zip(self._out_names, outs))
            futs.append((self._pool.submit(np.asarray, byname["UQ"]),
                         self._pool.submit(np.asarray, byname["US"])))
        out = np.empty((B, D), np.float32)
        for k, (fq, fs) in enumerate(futs):
            rows = slice(k * ROWS, (k + 1) * ROWS)
            uq, us = fq.result(), fs.result()
            np.multiply(uq, us.astype(np.float32), out=out[rows])
            np.multiply(out[rows], x0[rows], out=out[rows])
        return out


def _get_fast():
    if "fast" not in _CACHE:
        _CACHE["fast"] = _FastRunner()
    return _CACHE["fast"]


def _run_fallback(xq, sx, W2, x0):
    """Plain single-shot path via run_bass_kernel_spmd (full batch)."""
    from concourse.bass_utils import run_bass_kernel_spmd
    nc = _get_nc(BC)
    in_maps = []
    for c in range(NCORES):
        rows = slice(c * BC, (c + 1) * BC)
        in_maps.append({
            "XQ": xq[rows], "SX": sx[rows],
            "WSH": W2[c * WROWS_SH:(c + 1) * WROWS_SH],
        })
    res = run_bass_kernel_spmd(nc, in_maps, core_ids=list(range(NCORES)))
    out = np.empty((B, D), np.float32)
    for c in range(NCORES):
        rows = slice(c * BC, (c + 1) * BC)
        uq = res.results[c]["UQ"]
        us = res.results[c]["US"]
        np.multiply(uq, us.astype(np.float32), out=out[rows])
        np.multiply(out[rows], x0[rows], out=out[rows])
    return out


def kernel(inputs, U, V, C, G, bias):
    inputs = np.asarray(inputs, dtype=np.float32)
    U = np.asarray(U, dtype=np.float32)
    V = np.asarray(V, dtype=np.float32)
    C = np.asarray(C, dtype=np.float32)
    G = np.asarray(G, dtype=np.float32)
    bias = np.asarray(bias, dtype=np.float32)

    xq, sx = _quantize_x(inputs)
    W2 = _pack_weights(U, V, C, G, bias)
    try:
        return _get_fast().run(xq, sx, W2, inputs)
    except Exception:
        import sys, traceback
        traceback.print_exc()
        print("kernel: fast path failed; using fallback", file=sys.stderr)
        return _run_fallback(xq, sx, W2, inputs)


# revision 16
# speedup vs baseline: 5.5714x; 1.2190x over previous
"""CrossNetMix (moe_routing) Trainium2 Bass kernel — transfer-optimized.

Math (per layer i, softmax gates g sum to 1 over E):
    x_{l+1} = x_l + x0 * (sum_e g_e * U_e @ tanh(C_e @ tanh(V_e^T x_l)) + bias_i)

The residual chain collapses to x_L = x0 * u with
u = 1 + sum_i (gated_moe_i + bias_i), so the kernel carries u and
materializes y_i = y0 * u_i (transposed space y = x^T) as matmul input.

The axon host<->device tunnel moves ~46 MB/s TOTAL (shared across both
directions), so wall time ~= bytes moved / 46MB/s; on-device compute is
~0.2 ms.  Transfer plan (~36 MB per call vs 250 MB for the naive port):
  - x uploads as int8 with a per-row scale (16 MB + 32 KB): dequantized to
    fp16 on device, then PE-transposed into column-major y0.
  - the device returns U = u (not y = x0*u) quantized int8 with per-row
    scales computed on device; the host computes x0_f32 * (uq * su), so
    the final product uses EXACT x0 — x-quantization only enters through
    the layer nonlinearities (attenuated), keeping l2 error ~8e-3 vs the
    2e-2 gate.
  - all weights pack into one fp16 blob [128, 13112], sharded 8-ways
    (0.42 MB uploaded per core) and reassembled on device with a
    NeuronLink AllGather; constants (identity/select/ones) ship inside
    the NEFF via inline_tensor.
  - the batch runs as 4 pipeline slices whose downloads overlap later
    uploads; donated output buffers are created on device (no zeros
    upload); the weight blob is uploaded once per call and reused
    device-resident across slices.
Gating softmax stays in fp32 (fp16 exp would overflow for |logit| > 11);
the u accumulator stays fp32; matmuls take fp16 operands with fp32 PSUM
accumulation.  If the fast runner hits any problem, kernel() falls back
to a plain run_bass_kernel_spmd call on the same NEFF.
"""

import numpy as np

B, D, R, E, L = 16384, 1024, 64, 4, 3
NCORES = 8
BC = B // NCORES            # batch rows per core
BT = 512                    # batch columns per chunk (fp32 PSUM bank capacity)
KC = D // 128               # K-chunks over D
NM = D // 128               # M-chunks over D

NSLICE = 4                  # pipeline slices per kernel() call
BCS = BC // NSLICE          # rows per core per slice
ROWS = NCORES * BCS         # global rows per slice

QCLIP = 126.0               # int8 headroom so approx-reciprocal can't wrap

# packed fp16 weight blob [128, WCOLS] column offsets
OFF_GT = 0                  # [128, KC*E]        gating weights
OFF_BIA = OFF_GT + KC * E   # [128, L*NM]        bias (+1 folded into layer 0)
OFF_VP = OFF_BIA + L * NM   # L x [128, KC*2*128]
OFF_CB = OFF_VP + L * KC * 2 * 128   # L x [128, 2*128]
OFF_UP = OFF_CB + L * 2 * 128        # L x [128, 2*NM*128]
WCOLS = OFF_UP + L * 2 * NM * 128
WROWS_SH = 128 // NCORES    # blob rows uploaded per core

_CACHE = {}


def _build(bcs):
    import concourse.mybir as mybir
    import concourse.bacc as bacc
    import concourse.tile as tile

    nchunk = bcs // BT
    f32 = mybir.dt.float32
    f16 = mybir.dt.float16
    i8 = mybir.dt.int8
    ALU = mybir.AluOpType
    ACTF = mybir.ActivationFunctionType

    nc = bacc.Bacc("TRN2", target_bir_lowering=False, debug=False,
                   num_devices=NCORES)

    XQ = nc.dram_tensor("XQ", [bcs, D], i8, kind="ExternalInput")
    SX = nc.dram_tensor("SX", [bcs, 1], f16, kind="ExternalInput")
    WSH = nc.dram_tensor("WSH", [WROWS_SH, WCOLS], f16, kind="ExternalInput")
    UQ = nc.dram_tensor("UQ", [bcs, D], i8, kind="ExternalOutput")
    US = nc.dram_tensor("US", [bcs, 1], f16, kind="ExternalOutput")

    wbnc = nc.dram_tensor("wbnc", [WROWS_SH, WCOLS], f16)
    wall = nc.dram_tensor("wall", [128, WCOLS], f16, addr_space="Shared")

    # inline constants (shipped inside the NEFF, no upload)
    ident_h = np.eye(128, dtype=np.float16)
    sel_h = np.zeros((E, 2, 128), np.float32)
    for e in range(E):
        sel_h.reshape(E, 256)[e, e * 64:(e + 1) * 64] = 1.0
    ones_h = np.ones((E, E), np.float32)

    with tile.TileContext(nc) as tc:
        with (
            tc.tile_pool(name="wts", bufs=1) as wts,
            tc.tile_pool(name="xqp", bufs=2) as xqp,
            tc.tile_pool(name="xrp", bufs=2) as xrp,
            tc.tile_pool(name="sxp", bufs=2) as sxp,
            tc.tile_pool(name="y0p", bufs=3) as y0p,
            tc.tile_pool(name="yp", bufs=3) as yp,
            tc.tile_pool(name="up", bufs=2) as upool,
            tc.tile_pool(name="tp", bufs=2) as tp,
            tc.tile_pool(name="twp", bufs=2) as twp,
            tc.tile_pool(name="wgp", bufs=2) as wgp,
            tc.tile_pool(name="gp", bufs=2) as gp,
            tc.tile_pool(name="orp", bufs=2) as orp,
            tc.tile_pool(name="qsp", bufs=2) as qsp,
            tc.tile_pool(name="ps_g", bufs=1, space="PSUM") as ps_g,
            tc.tile_pool(name="ps_gbc", bufs=2, space="PSUM") as ps_gbc,
            tc.tile_pool(name="ps_vw", bufs=2, space="PSUM") as ps_vw,
            tc.tile_pool(name="ps_acc", bufs=2, space="PSUM") as ps_acc,
            tc.tile_pool(name="ps_tr", bufs=1, space="PSUM") as ps_tr,
        ):
            # gather the 8 weight shards over NeuronLink ASAP
            nc.sync.dma_start(out=wbnc[:, :], in_=WSH[:, :])
            nc.gpsimd.collective_compute(
                "AllGather", mybir.AluOpType.bypass,
                replica_groups=[list(range(NCORES))],
                ins=[wbnc.ap()], outs=[wall.ap()])

            id_sb = wts.tile([128, 128], f16, tag="id")
            nc.sync.dma_start(out=id_sb[:], in_=nc.inline_tensor(
                ident_h, name="ident")[:, :])
            sel_sb = wts.tile([E, 2, 128], f32, tag="sel")
            nc.sync.dma_start(out=sel_sb[:], in_=nc.inline_tensor(
                sel_h, name="sel")[:, :, :])
            ones_sb = wts.tile([E, E], f32, tag="ones")
            nc.sync.dma_start(out=ones_sb[:], in_=nc.inline_tensor(
                ones_h, name="ones")[:, :])

            # weight tiles from the gathered blob
            gt_sb = wts.tile([128, KC * E], f16, tag="gt")
            nc.sync.dma_start(out=gt_sb[:],
                              in_=wall[:, OFF_GT:OFF_GT + KC * E])
            bia16 = wts.tile([128, L * NM], f16, tag="bia16")
            nc.sync.dma_start(out=bia16[:],
                              in_=wall[:, OFF_BIA:OFF_BIA + L * NM])
            bia_sb = wts.tile([128, L * NM], f32, tag="bia")
            nc.scalar.activation(bia_sb[:], bia16[:], ACTF.Copy)
            vp_sb, cb_sb, up_sb = [], [], []
            for i in range(L):
                vp = wts.tile([128, KC * 2 * 128], f16, tag=f"vp{i}")
                nc.sync.dma_start(
                    out=vp[:], in_=wall[:, OFF_VP + i * KC * 256:
                                        OFF_VP + (i + 1) * KC * 256])
                vp_sb.append(vp)
                cb = wts.tile([128, 2 * 128], f16, tag=f"cb{i}")
                nc.sync.dma_start(
                    out=cb[:], in_=wall[:, OFF_CB + i * 256:
                                        OFF_CB + (i + 1) * 256])
                cb_sb.append(cb)
                up = wts.tile([128, 2 * NM * 128], f16, tag=f"up{i}")
                nc.sync.dma_start(
                    out=up[:], in_=wall[:, OFF_UP + i * NM * 256:
                                        OFF_UP + (i + 1) * NM * 256])
                up_sb.append(up)

            def load_chunk(cidx):
                """DMA 512 int8 rows, dequantize, transpose to y0."""
                r0 = cidx * BT
                xr = []
                for bi in range(4):
                    rs = slice(r0 + bi * 128, r0 + (bi + 1) * 128)
                    xq = xqp.tile([128, D], i8, tag=f"xq{bi}")
                    nc.sync.dma_start(out=xq[:], in_=XQ[rs, :])
                    sx16 = sxp.tile([128, 1], f16, tag=f"sx{bi}")
                    nc.sync.dma_start(out=sx16[:], in_=SX[rs, :])
                    sxf = sxp.tile([128, 1], f32, tag=f"sxf{bi}")
                    nc.scalar.activation(sxf[:], sx16[:], ACTF.Copy)
                    t_ = xrp.tile([128, D], f16, tag=f"xr{bi}")
                    nc.scalar.activation(t_[:], xq[:], ACTF.Copy,
                                         scale=sxf[:, 0:1])
                    xr.append(t_)
                y0 = y0p.tile([128, KC, BT], f16, tag="y0",
                              name=f"y0_{cidx}")
                for dj in range(KC):
                    pt = ps_tr.tile([128, D], f16, tag="tr", name="pt")
                    for bi in range(4):
                        nc.tensor.transpose(
                            pt[:, bi * 128:(bi + 1) * 128],
                            xr[bi][:, dj * 128:(dj + 1) * 128], id_sb[:])
                    nc.scalar.activation(y0[:, dj, :], pt[:, :BT], ACTF.Copy)
                return {"y0": y0, "y_in": y0, "u": None}

            def emit_layer(st, i):
                y0, y_in = st["y0"], st["y_in"]
                if i == 0:
                    st["u"] = upool.tile([128, NM, BT], f32, tag="u",
                                         name="u")
                u = st["u"]
                # --- S1 V-stage ---
                v_ps = [ps_vw.tile([128, BT], f32, tag="vw",
                                   name=f"v{pr_}") for pr_ in range(2)]
                for pr in range(2):
                    for k in range(KC):
                        nc.tensor.matmul(
                            v_ps[pr][:],
                            vp_sb[i][:, (k * 2 + pr) * 128:
                                     (k * 2 + pr + 1) * 128],
                            y_in[:, k, :],
                            start=(k == 0), stop=(k == KC - 1))
                # --- S0 gating logits ---
                glog = ps_g.tile([E, BT], f32, tag="g")
                for k in range(KC):
                    nc.tensor.matmul(glog[:], gt_sb[:, k * E:(k + 1) * E],
                                     y_in[:, k, :],
                                     start=(k == 0), stop=(k == KC - 1))
                t_sb = [tp.tile([128, BT], f16, tag="t",
                                name=f"t{pr_}") for pr_ in range(2)]
                for pr in range(2):
                    nc.scalar.activation(t_sb[pr][:], v_ps[pr][:], ACTF.Tanh)
                eg = gp.tile([E, BT], f32, tag="eg")
                nc.scalar.activation(eg[:], glog[:], ACTF.Exp)
                z4 = ps_g.tile([E, BT], f32, tag="g")
                nc.tensor.matmul(z4[:], ones_sb[:], eg[:], start=True,
                                 stop=True)
                rz4 = gp.tile([E, BT], f32, tag="rz", bufs=1)
                nc.vector.reciprocal_approx_fast(out=rz4[:], in_=z4[:])
                gn4 = gp.tile([E, BT], f32, tag="gn")
                nc.vector.tensor_mul(out=gn4[:], in0=eg[:], in1=rz4[:])
                # --- S2 C-stage ---
                w_ps = [ps_vw.tile([128, BT], f32, tag="vw",
                                   name=f"w{pr_}") for pr_ in range(2)]
                for pr in range(2):
                    nc.tensor.matmul(w_ps[pr][:],
                                     cb_sb[i][:, pr * 128:(pr + 1) * 128],
                                     t_sb[pr][:], start=True, stop=True)
                gbc_ps = [ps_gbc.tile([128, BT], f32, tag="gbc",
                                      name=f"gbc{pr_}") for pr_ in range(2)]
                for pr in range(2):
                    nc.tensor.matmul(gbc_ps[pr][:], sel_sb[:, pr, :],
                                     gn4[:], start=True, stop=True)
                wg_sb = []
                for pr in range(2):
                    tw = twp.tile([128, BT], f32, tag="tw")
                    nc.scalar.activation(tw[:], w_ps[pr][:], ACTF.Tanh)
                    wg = wgp.tile([128, BT], f16, tag="wg")
                    nc.vector.tensor_mul(out=wg[:], in0=tw[:],
                                         in1=gbc_ps[pr][:])
                    wg_sb.append(wg)
                # --- S3 U-stage + u update + y materialization ---
                last = (i == L - 1)
                if not last:
                    y_out = yp.tile([128, KC, BT], f16, tag="y")
                for m in range(NM):
                    acc = ps_acc.tile([128, BT], f32, tag="acc")
                    nc.tensor.matmul(acc[:],
                                     up_sb[i][:, m * 256:m * 256 + 128],
                                     wg_sb[0][:], start=True, stop=False)
                    nc.tensor.matmul(acc[:],
                                     up_sb[i][:, m * 256 + 128:m * 256 + 256],
                                     wg_sb[1][:], start=False, stop=True)
                    bcol = bia_sb[:, i * NM + m: i * NM + m + 1]
                    if i == 0:
                        # u = acc + (1 + bias_0)  (the 1+ is folded into BIA)
                        nc.scalar.activation(u[:, m, :], acc[:],
                                             ACTF.Identity, bias=bcol)
                    else:
                        nc.vector.scalar_tensor_tensor(
                            out=u[:, m, :], in0=acc[:], scalar=bcol,
                            in1=u[:, m, :], op0=ALU.add, op1=ALU.add)
                    if not last:
                        nc.vector.tensor_mul(out=y_out[:, m, :],
                                             in0=y0[:, m, :], in1=u[:, m, :])
                if not last:
                    st["y_in"] = y_out

            def store_chunk(st, cidx):
                """Quantize u int8 per batch row and DMA out with scales."""
                r0 = cidx * BT
                u = st["u"]
                uh = yp.tile([128, KC, BT], f16, tag="y", name="uh")
                for m in range(NM):
                    nc.scalar.activation(uh[:, m, :], u[:, m, :], ACTF.Copy)
                for bi in range(4):
                    rs = slice(r0 + bi * 128, r0 + (bi + 1) * 128)
                    po = ps_tr.tile([128, D], f16, tag="tr", name="po")
                    for dj in range(KC):
                        nc.tensor.transpose(
                            po[:, dj * 128:(dj + 1) * 128],
                            uh[:, dj, bi * 128:(bi + 1) * 128], id_sb[:])
                    m1 = qsp.tile([128, 1], f32, tag="m1")
                    nc.vector.tensor_reduce(
                        out=m1[:], in_=po[:, :],
                        axis=mybir.AxisListType.X,
                        op=ALU.max, apply_absolute_value=True)
                    nc.vector.tensor_scalar_max(out=m1[:], in0=m1[:],
                                                scalar1=1e-6)
                    rq = qsp.tile([128, 1], f32, tag="rq")
                    nc.vector.reciprocal(out=rq[:], in_=m1[:])
                    nc.vector.tensor_scalar_mul(out=rq[:], in0=rq[:],
                                                scalar1=QCLIP)
                    qo = orp.tile([128, D], i8, tag="or")
                    nc.scalar.activation(qo[:], po[:], ACTF.Copy,
                                         scale=rq[:, 0:1])
                    us16 = qsp.tile([128, 1], f16, tag="us")
                    nc.scalar.activation(us16[:], m1[:], ACTF.Copy,
                                         scale=1.0 / QCLIP)
                    nc.sync.dma_start(out=UQ[rs, :], in_=qo[:])
                    nc.sync.dma_start(out=US[rs, :], in_=us16[:])

            for cidx in range(nchunk):
                st = load_chunk(cidx)
                for i in range(L):
                    emit_layer(st, i)
                store_chunk(st, cidx)
    nc.compile()
    return nc


def _get_nc(bcs):
    key = ("nc", bcs)
    if key not in _CACHE:
        _CACHE[key] = _build(bcs)
    return _CACHE[key]


def _pack_weights(U, V, C, G, bias):
    """One fp16 blob [128, WCOLS] holding every weight in its SBUF layout."""
    W2 = np.zeros((128, WCOLS), np.float16)
    # gating [128, KC*E]: G.T [D, E] -> [KC, 128, E] -> [128, KC, E]
    W2[:, OFF_GT:OFF_GT + KC * E] = (
        G.T.reshape(KC, 128, E).transpose(1, 0, 2).reshape(128, KC * E))
    # bias [128, L*NM] with the residual "1 +" folded into layer 0
    biasm = bias.astype(np.float32, copy=True)
    biasm[0] += 1.0
    W2[:, OFF_BIA:OFF_BIA + L * NM] = (
        biasm.reshape(L, NM, 128).transpose(2, 0, 1).reshape(128, L * NM))
    # V packed pairs: [L, KC, 128, 2, 128] -> per layer [128, KC*2*128]
    VPh = V.transpose(0, 2, 1, 3).reshape(L, D, E * R).reshape(
        L, KC, 128, 2, 128)
    for i in range(L):
        W2[:, OFF_VP + i * KC * 256:OFF_VP + (i + 1) * KC * 256] = (
            VPh[i].transpose(1, 0, 2, 3).reshape(128, KC * 256))
    # C block-diagonal transposed: [L, 2, 128, 128] -> [128, 2*128]
    CBh = np.zeros((L, 2, 128, 128), np.float32)
    for i in range(L):
        for pr in range(2):
            CBh[i, pr, :64, :64] = C[i, 2 * pr].T
            CBh[i, pr, 64:, 64:] = C[i, 2 * pr + 1].T
    for i in range(L):
        W2[:, OFF_CB + i * 256:OFF_CB + (i + 1) * 256] = (
            CBh[i].transpose(1, 0, 2).reshape(128, 256))
    # U packed: [L, 2, 128, NM, 128] -> per layer [128, NM, 2, 128] flat,
    # matching the kernel's [:, m*256 + pr*128 + col] indexing
    UPh = U.transpose(0, 1, 3, 2).reshape(L, E * R, D).reshape(
        L, 2, 128, NM, 128)
    for i in range(L):
        W2[:, OFF_UP + i * NM * 256:OFF_UP + (i + 1) * NM * 256] = (
            UPh[i].transpose(1, 2, 0, 3).reshape(128, NM * 256))
    return W2


def _quantize_x(x):
    """Per-row symmetric int8: returns (int8 [n,D], fp16 scales [n,1])."""
    sx = np.abs(x).max(axis=1)
    np.maximum(sx, 1e-20, out=sx)
    q = x * (QCLIP / sx)[:, None]
    np.rint(q, out=q)
    return q.astype(np.int8), (sx * (1.0 / QCLIP)).astype(np.float16)[:, None]


class _FastRunner:
    """Pipelined multi-slice executor built on the bass_exec PJRT primitive.

    Mirrors run_bass_via_pjrt's binding protocol exactly, but keeps the
    jitted executable cached, keeps the weight blob device-resident across
    slices, creates the donated output buffers on device, and overlaps
    slice k's download with slice k+1's upload (the tunnel is full-duplex).
    """

    def __init__(self):
        import concurrent.futures as cf
        import jax
        import concourse.mybir as mybir
        from jax.experimental.shard_map import shard_map
        from jax.sharding import Mesh, PartitionSpec, NamedSharding
        from concourse import bass2jax

        nc = _get_nc(BCS)
        if nc.dbg_addr is not None or nc.dbg_callbacks:
            raise RuntimeError("fast runner needs debug=False")
        bass2jax.install_neuronx_cc_hook()
        self._jax = jax

        partition_name = (nc.partition_id_tensor.name
                          if nc.partition_id_tensor else None)
        in_names, out_names, out_avals = [], [], []
        for alloc in nc.m.functions[0].allocations:
            if not isinstance(alloc, mybir.MemoryLocationSet):
                continue
            name = alloc.memorylocations[0].name
            if alloc.kind == "ExternalInput":
                if name != partition_name:
                    in_names.append(name)
            elif alloc.kind == "ExternalOutput":
                shape = tuple(alloc.tensor_shape)
                dtype = mybir.dt.np(alloc.dtype)
                out_names.append(name)
                out_avals.append(jax.core.ShapedArray(shape, dtype))
        assert set(in_names) == {"XQ", "SX", "WSH"}, in_names
        assert set(out_names) == {"UQ", "US"}, out_names
        n_params = len(in_names)
        self._arg_names = list(in_names)
        self._out_names = list(out_names)
        all_names = in_names + out_names
        if partition_name is not None:
            all_names = all_names + [partition_name]

        def _body(*args):
            operands = list(args)
            if partition_name is not None:
                operands.append(bass2jax.partition_id_tensor())
            outs = bass2jax._bass_exec_p.bind(
                *operands,
                out_avals=tuple(out_avals),
                in_names=tuple(all_names),
                out_names=tuple(out_names),
                lowering_input_output_aliases=(),
                sim_require_finite=True,
                sim_require_nnan=True,
                nc=nc,
            )
            return tuple(outs)

        devices = jax.devices()[:NCORES]
        assert len(devices) == NCORES, devices
        mesh = Mesh(np.asarray(devices), ("core",))
        self._sharding = NamedSharding(mesh, PartitionSpec("core"))
        n_all = n_params + len(out_names)
        in_specs = (PartitionSpec("core"),) * n_all
        out_specs = (PartitionSpec("core"),) * len(out_names)
        donate = tuple(range(n_params, n_all))
        self._jitted = jax.jit(
            shard_map(_body, mesh=mesh, in_specs=in_specs,
                      out_specs=out_specs, check_rep=False),
            donate_argnums=donate, keep_unused=True)
        zshapes = [((NCORES * a.shape[0],) + a.shape[1:], a.dtype)
                   for a in out_avals]
        self._zjit = jax.jit(
            lambda: tuple(jax.numpy.zeros(s, d) for s, d in zshapes),
            out_shardings=tuple(self._sharding for _ in zshapes))
        self._pool = cf.ThreadPoolExecutor(max_workers=2)
        self._wkey = None
        self._wdev = None

    def get_wdev(self, U, V, C, G, bias):
        """Device-resident weight blob, reused across calls when unchanged."""
        parts = (U, V, C, G, bias)
        if self._wkey is not None and all(
                np.array_equal(a, b) for a, b in zip(self._wkey, parts)):
            return self._wdev
        W2 = _pack_weights(U, V, C, G, bias)
        self._wdev = self._jax.device_put(W2, self._sharding)
        self._wkey = tuple(p.copy() for p in parts)
        return self._wdev

    def run(self, wdev, x0):
        """Returns the final f32 output [B, D] = x0 * dequant(u).

        Quantizes each slice on the host just before its upload, so host
        work overlaps earlier slices' transfers; likewise the dequant of
        slice k overlaps slice k+1's download.
        """
        jax = self._jax
        futs = []
        for k in range(NSLICE):
            rows = slice(k * ROWS, (k + 1) * ROWS)
            xq_k, sx_k = _quantize_x(x0[rows])
            ops = {"XQ": jax.device_put(xq_k, self._sharding),
                   "SX": jax.device_put(sx_k, self._sharding),
                   "WSH": wdev}
            args = tuple(ops[n] for n in self._arg_names)
            outs = self._jitted(*args, *self._zjit())
            byname = dict(zip(self._out_names, outs))
            futs.append((self._pool.submit(np.asarray, byname["UQ"]),
                         self._pool.submit(np.asarray, byname["US"])))
        out = np.empty((B, D), np.float32)
        for k, (fq, fs) in enumerate(futs):
            rows = slice(k * ROWS, (k + 1) * ROWS)
            uq, us = fq.result(), fs.result()
            np.multiply(uq, us.astype(np.float32), out=out[rows])
            np.multiply(out[rows], x0[rows], out=out[rows])
        return out


def _get_fast():
    if "fast" not in _CACHE:
        _CACHE["fast"] = _FastRunner()
    return _CACHE["fast"]


def _run_fallback(xq, sx, W2, x0):
    """Plain single-shot path via run_bass_kernel_spmd (full batch)."""
    from concourse.bass_utils import run_bass_kernel_spmd
    nc = _get_nc(BC)
    in_maps = []
    for c in range(NCORES):
        rows = slice(c * BC, (c + 1) * BC)
        in_maps.append({
            "XQ": xq[rows], "SX": sx[rows],
            "WSH": W2[c * WROWS_SH:(c + 1) * WROWS_SH],
        })
    res = run_bass_kernel_spmd(nc, in_maps, core_ids=list(range(NCORES)))
    out = np.empty((B, D), np.float32)
    for c in range(NCORES):
        rows = slice(c * BC, (c + 1) * BC)
        uq = res.results[c]["UQ"]
        us = res.results[c]["US"]
        np.multiply(uq, us.astype(np.float32), out=out[rows])
        np.multiply(out[rows], x0[rows], out=out[rows])
    return out


def kernel(inputs, U, V, C, G, bias):
    inputs = np.asarray(inputs, dtype=np.float32)
    U = np.asarray(U, dtype=np.float32)
    V = np.asarray(V, dtype=np.float32)
    C = np.asarray(C, dtype=np.float32)
    G = np.asarray(G, dtype=np.float32)
    bias = np.asarray(bias, dtype=np.float32)

    try:
        fast = _get_fast()
        return fast.run(fast.get_wdev(U, V, C, G, bias), inputs)
    except Exception:
        import sys, traceback
        traceback.print_exc()
        print("kernel: fast path failed; using fallback", file=sys.stderr)
        xq, sx = _quantize_x(inputs)
        W2 = _pack_weights(U, V, C, G, bias)
        return _run_fallback(xq, sx, W2, inputs)


# revision 26
# speedup vs baseline: 5.9649x; 1.0706x over previous
"""CrossNetMix (moe_routing) Trainium2 Bass kernel — transfer-optimized.

Math (per layer i, softmax gates g sum to 1 over E):
    x_{l+1} = x_l + x0 * (sum_e g_e * U_e @ tanh(C_e @ tanh(V_e^T x_l)) + bias_i)

The residual chain collapses to x_L = x0 * u with
u = 1 + sum_i (gated_moe_i + bias_i), so the kernel carries u and
materializes y_i = y0 * u_i (transposed space y = x^T) as matmul input.

The axon host<->device tunnel is the whole game: ~50 MB/s per direction
for large transfers, degrading to ~46 MB/s AGGREGATE when both directions
run concurrently — so overlapping upload with download does not reduce
byte-time, and a single full-batch call beats sliced pipelining (measured:
1 slice 0.86s, 2 slices 0.93s, 4 slices 1.01s, 8 slices 1.40s).  On-device
compute is ~0.2 ms.  Transfer plan (~32 MB per call vs ~250 MB naive):
  - x uploads as int8 with a per-row scale (16 MB + 32 KB): dequantized to
    fp16 on device, then PE-transposed into column-major y0.
  - the device returns u (not y = x0*u) quantized int8 with per-row
    scales computed on device via abs-max reduce; the host computes
    x0_f32 * (uq * su), so the final product uses EXACT x0 —
    x-quantization only enters through the layer nonlinearities
    (attenuated), keeping l2 error ~4.8e-3 vs the 2e-2 gate.
  - all weights pack into one fp16 blob [128, 13112], sharded 8-ways
    (0.42 MB uploaded per core) and reassembled on device with a
    NeuronLink AllGather; constants (identity/select/ones) ship inside
    the NEFF via inline_tensor; the blob upload is skipped entirely when
    the weights are unchanged from the previous call (device-resident).
  - donated output buffers are created on device (no zeros upload); host
    quantize/dequantize run banded across 4 threads.
Gating softmax stays in fp32 (fp16 exp would overflow for |logit| > 11);
the u accumulator stays fp32; matmuls take fp16 operands with fp32 PSUM
accumulation.  If the fast runner hits any problem, kernel() falls back
to a plain run_bass_kernel_spmd call on the same NEFF.
"""

import numpy as np

B, D, R, E, L = 16384, 1024, 64, 4, 3
NCORES = 8
BC = B // NCORES            # batch rows per core
BT = 512                    # batch columns per chunk (fp32 PSUM bank capacity)
KC = D // 128               # K-chunks over D
NM = D // 128               # M-chunks over D

NSLICE = 1                  # single-shot: see transfer notes in docstring
BCS = BC // NSLICE          # rows per core per slice
ROWS = NCORES * BCS         # global rows per slice

QCLIP = 126.0               # int8 headroom so approx-reciprocal can't wrap

# packed fp16 weight blob [128, WCOLS] column offsets
OFF_GT = 0                  # [128, KC*E]        gating weights
OFF_BIA = OFF_GT + KC * E   # [128, L*NM]        bias (+1 folded into layer 0)
OFF_VP = OFF_BIA + L * NM   # L x [128, KC*2*128]
OFF_CB = OFF_VP + L * KC * 2 * 128   # L x [128, 2*128]
OFF_UP = OFF_CB + L * 2 * 128        # L x [128, 2*NM*128]
WCOLS = OFF_UP + L * 2 * NM * 128
WROWS_SH = 128 // NCORES    # blob rows uploaded per core

_CACHE = {}


def _build(bcs):
    import concourse.mybir as mybir
    import concourse.bacc as bacc
    import concourse.tile as tile

    bt = min(BT, bcs)       # batch columns per chunk
    nchunk = bcs // bt
    nbi = bt // 128         # 128-row blocks per chunk
    f32 = mybir.dt.float32
    f16 = mybir.dt.float16
    i8 = mybir.dt.int8
    ALU = mybir.AluOpType
    ACTF = mybir.ActivationFunctionType

    nc = bacc.Bacc("TRN2", target_bir_lowering=False, debug=False,
                   num_devices=NCORES)

    XQ = nc.dram_tensor("XQ", [bcs, D], i8, kind="ExternalInput")
    SX = nc.dram_tensor("SX", [bcs, 1], f16, kind="ExternalInput")
    WSH = nc.dram_tensor("WSH", [WROWS_SH, WCOLS], f16, kind="ExternalInput")
    UQ = nc.dram_tensor("UQ", [bcs, D], i8, kind="ExternalOutput")
    US = nc.dram_tensor("US", [bcs, 1], f16, kind="ExternalOutput")

    wbnc = nc.dram_tensor("wbnc", [WROWS_SH, WCOLS], f16)
    wall = nc.dram_tensor("wall", [128, WCOLS], f16, addr_space="Shared")

    # inline constants (shipped inside the NEFF, no upload)
    ident_h = np.eye(128, dtype=np.float16)
    sel_h = np.zeros((E, 2, 128), np.float32)
    for e in range(E):
        sel_h.reshape(E, 256)[e, e * 64:(e + 1) * 64] = 1.0
    ones_h = np.ones((E, E), np.float32)

    with tile.TileContext(nc) as tc:
        with (
            tc.tile_pool(name="wts", bufs=1) as wts,
            tc.tile_pool(name="xqp", bufs=2) as xqp,
            tc.tile_pool(name="xrp", bufs=2) as xrp,
            tc.tile_pool(name="sxp", bufs=2) as sxp,
            tc.tile_pool(name="y0p", bufs=3) as y0p,
            tc.tile_pool(name="yp", bufs=3) as yp,
            tc.tile_pool(name="up", bufs=2) as upool,
            tc.tile_pool(name="tp", bufs=2) as tp,
            tc.tile_pool(name="twp", bufs=2) as twp,
            tc.tile_pool(name="wgp", bufs=2) as wgp,
            tc.tile_pool(name="gp", bufs=2) as gp,
            tc.tile_pool(name="orp", bufs=2) as orp,
            tc.tile_pool(name="qsp", bufs=2) as qsp,
            tc.tile_pool(name="ps_g", bufs=1, space="PSUM") as ps_g,
            tc.tile_pool(name="ps_gbc", bufs=2, space="PSUM") as ps_gbc,
            tc.tile_pool(name="ps_vw", bufs=2, space="PSUM") as ps_vw,
            tc.tile_pool(name="ps_acc", bufs=2, space="PSUM") as ps_acc,
            tc.tile_pool(name="ps_tr", bufs=1, space="PSUM") as ps_tr,
        ):
            # gather the 8 weight shards over NeuronLink ASAP
            nc.sync.dma_start(out=wbnc[:, :], in_=WSH[:, :])
            nc.gpsimd.collective_compute(
                "AllGather", mybir.AluOpType.bypass,
                replica_groups=[list(range(NCORES))],
                ins=[wbnc.ap()], outs=[wall.ap()])

            id_sb = wts.tile([128, 128], f16, tag="id")
            nc.sync.dma_start(out=id_sb[:], in_=nc.inline_tensor(
                ident_h, name="ident")[:, :])
            sel_sb = wts.tile([E, 2, 128], f32, tag="sel")
            nc.sync.dma_start(out=sel_sb[:], in_=nc.inline_tensor(
                sel_h, name="sel")[:, :, :])
            ones_sb = wts.tile([E, E], f32, tag="ones")
            nc.sync.dma_start(out=ones_sb[:], in_=nc.inline_tensor(
                ones_h, name="ones")[:, :])

            # weight tiles from the gathered blob
            gt_sb = wts.tile([128, KC * E], f16, tag="gt")
            nc.sync.dma_start(out=gt_sb[:],
                              in_=wall[:, OFF_GT:OFF_GT + KC * E])
            bia16 = wts.tile([128, L * NM], f16, tag="bia16")
            nc.sync.dma_start(out=bia16[:],
                              in_=wall[:, OFF_BIA:OFF_BIA + L * NM])
            bia_sb = wts.tile([128, L * NM], f32, tag="bia")
            nc.scalar.activation(bia_sb[:], bia16[:], ACTF.Copy)
            vp_sb, cb_sb, up_sb = [], [], []
            for i in range(L):
                vp = wts.tile([128, KC * 2 * 128], f16, tag=f"vp{i}")
                nc.sync.dma_start(
                    out=vp[:], in_=wall[:, OFF_VP + i * KC * 256:
                                        OFF_VP + (i + 1) * KC * 256])
                vp_sb.append(vp)
                cb = wts.tile([128, 2 * 128], f16, tag=f"cb{i}")
                nc.sync.dma_start(
                    out=cb[:], in_=wall[:, OFF_CB + i * 256:
                                        OFF_CB + (i + 1) * 256])
                cb_sb.append(cb)
                up = wts.tile([128, 2 * NM * 128], f16, tag=f"up{i}")
                nc.sync.dma_start(
                    out=up[:], in_=wall[:, OFF_UP + i * NM * 256:
                                        OFF_UP + (i + 1) * NM * 256])
                up_sb.append(up)

            def load_chunk(cidx):
                """DMA 512 int8 rows, dequantize, transpose to y0."""
                r0 = cidx * bt
                xr = []
                for bi in range(nbi):
                    rs = slice(r0 + bi * 128, r0 + (bi + 1) * 128)
                    xq = xqp.tile([128, D], i8, tag=f"xq{bi}")
                    nc.sync.dma_start(out=xq[:], in_=XQ[rs, :])
                    sx16 = sxp.tile([128, 1], f16, tag=f"sx{bi}")
                    nc.sync.dma_start(out=sx16[:], in_=SX[rs, :])
                    sxf = sxp.tile([128, 1], f32, tag=f"sxf{bi}")
                    nc.scalar.activation(sxf[:], sx16[:], ACTF.Copy)
                    t_ = xrp.tile([128, D], f16, tag=f"xr{bi}")
                    nc.scalar.activation(t_[:], xq[:], ACTF.Copy,
                                         scale=sxf[:, 0:1])
                    xr.append(t_)
                y0 = y0p.tile([128, KC, bt], f16, tag="y0",
                              name=f"y0_{cidx}")
                for dj in range(KC):
                    pt = ps_tr.tile([128, D], f16, tag="tr", name="pt")
                    for bi in range(nbi):
                        nc.tensor.transpose(
                            pt[:, bi * 128:(bi + 1) * 128],
                            xr[bi][:, dj * 128:(dj + 1) * 128], id_sb[:])
                    nc.scalar.activation(y0[:, dj, :], pt[:, :bt], ACTF.Copy)
                return {"y0": y0, "y_in": y0, "u": None}

            def emit_layer(st, i):
                y0, y_in = st["y0"], st["y_in"]
                if i == 0:
                    st["u"] = upool.tile([128, NM, bt], f32, tag="u",
                                         name="u")
                u = st["u"]
                # --- S1 V-stage ---
                v_ps = [ps_vw.tile([128, bt], f32, tag="vw",
                                   name=f"v{pr_}") for pr_ in range(2)]
                for pr in range(2):
                    for k in range(KC):
                        nc.tensor.matmul(
                            v_ps[pr][:],
                            vp_sb[i][:, (k * 2 + pr) * 128:
                                     (k * 2 + pr + 1) * 128],
                            y_in[:, k, :],
                            start=(k == 0), stop=(k == KC - 1))
                # --- S0 gating logits ---
                glog = ps_g.tile([E, bt], f32, tag="g")
                for k in range(KC):
                    nc.tensor.matmul(glog[:], gt_sb[:, k * E:(k + 1) * E],
                                     y_in[:, k, :],
                                     start=(k == 0), stop=(k == KC - 1))
                t_sb = [tp.tile([128, bt], f16, tag="t",
                                name=f"t{pr_}") for pr_ in range(2)]
                for pr in range(2):
                    nc.scalar.activation(t_sb[pr][:], v_ps[pr][:], ACTF.Tanh)
                eg = gp.tile([E, bt], f32, tag="eg")
                nc.scalar.activation(eg[:], glog[:], ACTF.Exp)
                z4 = ps_g.tile([E, bt], f32, tag="g")
                nc.tensor.matmul(z4[:], ones_sb[:], eg[:], start=True,
                                 stop=True)
                rz4 = gp.tile([E, bt], f32, tag="rz", bufs=1)
                nc.vector.reciprocal_approx_fast(out=rz4[:], in_=z4[:])
                gn4 = gp.tile([E, bt], f32, tag="gn")
                nc.vector.tensor_mul(out=gn4[:], in0=eg[:], in1=rz4[:])
                # --- S2 C-stage ---
                w_ps = [ps_vw.tile([128, bt], f32, tag="vw",
                                   name=f"w{pr_}") for pr_ in range(2)]
                for pr in range(2):
                    nc.tensor.matmul(w_ps[pr][:],
                                     cb_sb[i][:, pr * 128:(pr + 1) * 128],
                                     t_sb[pr][:], start=True, stop=True)
                gbc_ps = [ps_gbc.tile([128, bt], f32, tag="gbc",
                                      name=f"gbc{pr_}") for pr_ in range(2)]
                for pr in range(2):
                    nc.tensor.matmul(gbc_ps[pr][:], sel_sb[:, pr, :],
                                     gn4[:], start=True, stop=True)
                wg_sb = []
                for pr in range(2):
                    tw = twp.tile([128, bt], f32, tag="tw")
                    nc.scalar.activation(tw[:], w_ps[pr][:], ACTF.Tanh)
                    wg = wgp.tile([128, bt], f16, tag="wg")
                    nc.vector.tensor_mul(out=wg[:], in0=tw[:],
                                         in1=gbc_ps[pr][:])
                    wg_sb.append(wg)
                # --- S3 U-stage + u update + y materialization ---
                last = (i == L - 1)
                if not last:
                    y_out = yp.tile([128, KC, bt], f16, tag="y")
                for m in range(NM):
                    acc = ps_acc.tile([128, bt], f32, tag="acc")
                    nc.tensor.matmul(acc[:],
                                     up_sb[i][:, m * 256:m * 256 + 128],
                                     wg_sb[0][:], start=True, stop=False)
                    nc.tensor.matmul(acc[:],
                                     up_sb[i][:, m * 256 + 128:m * 256 + 256],
                                     wg_sb[1][:], start=False, stop=True)
                    bcol = bia_sb[:, i * NM + m: i * NM + m + 1]
                    if i == 0:
                        # u = acc + (1 + bias_0)  (the 1+ is folded into BIA)
                        nc.scalar.activation(u[:, m, :], acc[:],
                                             ACTF.Identity, bias=bcol)
                    else:
                        nc.vector.scalar_tensor_tensor(
                            out=u[:, m, :], in0=acc[:], scalar=bcol,
                            in1=u[:, m, :], op0=ALU.add, op1=ALU.add)
                    if not last:
                        nc.vector.tensor_mul(out=y_out[:, m, :],
                                             in0=y0[:, m, :], in1=u[:, m, :])
                if not last:
                    st["y_in"] = y_out

            def store_chunk(st, cidx):
                """Quantize u int8 per batch row and DMA out with scales."""
                r0 = cidx * bt
                u = st["u"]
                uh = yp.tile([128, KC, bt], f16, tag="y", name="uh")
                for m in range(NM):
                    nc.scalar.activation(uh[:, m, :], u[:, m, :], ACTF.Copy)
                for bi in range(nbi):
                    rs = slice(r0 + bi * 128, r0 + (bi + 1) * 128)
                    po = ps_tr.tile([128, D], f16, tag="tr", name="po")
                    for dj in range(KC):
                        nc.tensor.transpose(
                            po[:, dj * 128:(dj + 1) * 128],
                            uh[:, dj, bi * 128:(bi + 1) * 128], id_sb[:])
                    m1 = qsp.tile([128, 1], f32, tag="m1")
                    nc.vector.tensor_reduce(
                        out=m1[:], in_=po[:, :],
                        axis=mybir.AxisListType.X,
                        op=ALU.max, apply_absolute_value=True)
                    nc.vector.tensor_scalar_max(out=m1[:], in0=m1[:],
                                                scalar1=1e-6)
                    rq = qsp.tile([128, 1], f32, tag="rq")
                    nc.vector.reciprocal(out=rq[:], in_=m1[:])
                    nc.vector.tensor_scalar_mul(out=rq[:], in0=rq[:],
                                                scalar1=QCLIP)
                    qo = orp.tile([128, D], i8, tag="or")
                    nc.scalar.activation(qo[:], po[:], ACTF.Copy,
                                         scale=rq[:, 0:1])
                    us16 = qsp.tile([128, 1], f16, tag="us")
                    nc.scalar.activation(us16[:], m1[:], ACTF.Copy,
                                         scale=1.0 / QCLIP)
                    nc.sync.dma_start(out=UQ[rs, :], in_=qo[:])
                    nc.sync.dma_start(out=US[rs, :], in_=us16[:])

            for cidx in range(nchunk):
                st = load_chunk(cidx)
                for i in range(L):
                    emit_layer(st, i)
                store_chunk(st, cidx)
    nc.compile()
    return nc


def _get_nc(bcs):
    key = ("nc", bcs)
    if key not in _CACHE:
        _CACHE[key] = _build(bcs)
    return _CACHE[key]


def _pack_weights(U, V, C, G, bias):
    """One fp16 blob [128, WCOLS] holding every weight in its SBUF layout."""
    W2 = np.zeros((128, WCOLS), np.float16)
    # gating [128, KC*E]: G.T [D, E] -> [KC, 128, E] -> [128, KC, E]
    W2[:, OFF_GT:OFF_GT + KC * E] = (
        G.T.reshape(KC, 128, E).transpose(1, 0, 2).reshape(128, KC * E))
    # bias [128, L*NM] with the residual "1 +" folded into layer 0
    biasm = bias.astype(np.float32, copy=True)
    biasm[0] += 1.0
    W2[:, OFF_BIA:OFF_BIA + L * NM] = (
        biasm.reshape(L, NM, 128).transpose(2, 0, 1).reshape(128, L * NM))
    # V packed pairs: [L, KC, 128, 2, 128] -> per layer [128, KC*2*128]
    VPh = V.transpose(0, 2, 1, 3).reshape(L, D, E * R).reshape(
        L, KC, 128, 2, 128)
    for i in range(L):
        W2[:, OFF_VP + i * KC * 256:OFF_VP + (i + 1) * KC * 256] = (
            VPh[i].transpose(1, 0, 2, 3).reshape(128, KC * 256))
    # C block-diagonal transposed: [L, 2, 128, 128] -> [128, 2*128]
    CBh = np.zeros((L, 2, 128, 128), np.float32)
    for i in range(L):
        for pr in range(2):
            CBh[i, pr, :64, :64] = C[i, 2 * pr].T
            CBh[i, pr, 64:, 64:] = C[i, 2 * pr + 1].T
    for i in range(L):
        W2[:, OFF_CB + i * 256:OFF_CB + (i + 1) * 256] = (
            CBh[i].transpose(1, 0, 2).reshape(128, 256))
    # U packed: [L, 2, 128, NM, 128] -> per layer [128, NM, 2, 128] flat,
    # matching the kernel's [:, m*256 + pr*128 + col] indexing
    UPh = U.transpose(0, 1, 3, 2).reshape(L, E * R, D).reshape(
        L, 2, 128, NM, 128)
    for i in range(L):
        W2[:, OFF_UP + i * NM * 256:OFF_UP + (i + 1) * NM * 256] = (
            UPh[i].transpose(1, 2, 0, 3).reshape(128, NM * 256))
    return W2


_HPOOL = None


def _hpool():
    """Thread pool for banded numpy work (ufuncs release the GIL)."""
    global _HPOOL
    if _HPOOL is None:
        import concurrent.futures as cf
        _HPOOL = cf.ThreadPoolExecutor(max_workers=4)
    return _HPOOL


def _banded(n, fn, bands=4):
    step = -(-n // bands)
    step = -(-step // 128) * 128
    futs = [_hpool().submit(fn, b0, min(b0 + step, n))
            for b0 in range(0, n, step)]
    for f in futs:
        f.result()


def _quantize_x(x):
    """Per-row symmetric int8: returns (int8 [n,D], fp16 scales [n,1])."""
    n = x.shape[0]
    q = np.empty(x.shape, np.int8)
    s = np.empty((n, 1), np.float16)

    def band(b0, b1):
        xb = x[b0:b1]
        sx = np.abs(xb).max(axis=1)
        np.maximum(sx, 1e-20, out=sx)
        t = xb * (QCLIP / sx)[:, None]
        np.rint(t, out=t)
        q[b0:b1] = t
        s[b0:b1, 0] = sx * (1.0 / QCLIP)

    _banded(n, band)
    return q, s


def _decode_u(out, r0, uq, us, x0):
    """out[r0:r0+n] = x0[r0:r0+n] * (uq * us), banded across threads."""
    n = uq.shape[0]
    usf = us.astype(np.float32)

    def band(b0, b1):
        np.multiply(uq[b0:b1], usf[b0:b1], out=out[r0 + b0:r0 + b1])
        np.multiply(out[r0 + b0:r0 + b1], x0[r0 + b0:r0 + b1],
                    out=out[r0 + b0:r0 + b1])

    _banded(n, band)


class _FastRunner:
    """Low-overhead executor built on the bass_exec PJRT primitive.

    Mirrors run_bass_via_pjrt's binding protocol exactly, but keeps the
    jitted executable cached across calls, keeps the weight blob
    device-resident (re-uploaded only when the weights change), and
    creates the donated output buffers on device instead of uploading
    host zeros.
    """

    def __init__(self):
        import concurrent.futures as cf
        import jax
        import concourse.mybir as mybir
        from jax.experimental.shard_map import shard_map
        from jax.sharding import Mesh, PartitionSpec, NamedSharding
        from concourse import bass2jax

        nc = _get_nc(BCS)
        if nc.dbg_addr is not None or nc.dbg_callbacks:
            raise RuntimeError("fast runner needs debug=False")
        bass2jax.install_neuronx_cc_hook()
        self._jax = jax

        partition_name = (nc.partition_id_tensor.name
                          if nc.partition_id_tensor else None)
        in_names, out_names, out_avals = [], [], []
        for alloc in nc.m.functions[0].allocations:
            if not isinstance(alloc, mybir.MemoryLocationSet):
                continue
            name = alloc.memorylocations[0].name
            if alloc.kind == "ExternalInput":
                if name != partition_name:
                    in_names.append(name)
            elif alloc.kind == "ExternalOutput":
                shape = tuple(alloc.tensor_shape)
                dtype = mybir.dt.np(alloc.dtype)
                out_names.append(name)
                out_avals.append(jax.core.ShapedArray(shape, dtype))
        assert set(in_names) == {"XQ", "SX", "WSH"}, in_names
        assert set(out_names) == {"UQ", "US"}, out_names
        n_params = len(in_names)
        self._arg_names = list(in_names)
        self._out_names = list(out_names)
        all_names = in_names + out_names
        if partition_name is not None:
            all_names = all_names + [partition_name]

        def _body(*args):
            operands = list(args)
            if partition_name is not None:
                operands.append(bass2jax.partition_id_tensor())
            outs = bass2jax._bass_exec_p.bind(
                *operands,
                out_avals=tuple(out_avals),
                in_names=tuple(all_names),
                out_names=tuple(out_names),
                lowering_input_output_aliases=(),
                sim_require_finite=True,
                sim_require_nnan=True,
                nc=nc,
            )
            return tuple(outs)

        devices = jax.devices()[:NCORES]
        assert len(devices) == NCORES, devices
        mesh = Mesh(np.asarray(devices), ("core",))
        self._sharding = NamedSharding(mesh, PartitionSpec("core"))
        n_all = n_params + len(out_names)
        in_specs = (PartitionSpec("core"),) * n_all
        out_specs = (PartitionSpec("core"),) * len(out_names)
        donate = tuple(range(n_params, n_all))
        self._jitted = jax.jit(
            shard_map(_body, mesh=mesh, in_specs=in_specs,
                      out_specs=out_specs, check_rep=False),
            donate_argnums=donate, keep_unused=True)
        zshapes = [((NCORES * a.shape[0],) + a.shape[1:], a.dtype)
                   for a in out_avals]
        self._zjit = jax.jit(
            lambda: tuple(jax.numpy.zeros(s, d) for s, d in zshapes),
            out_shardings=tuple(self._sharding for _ in zshapes))
        self._pool = cf.ThreadPoolExecutor(max_workers=2)
        self._wkey = None
        self._wdev = None

    def get_wdev(self, U, V, C, G, bias):
        """Device-resident weight blob, reused across calls when unchanged."""
        parts = (U, V, C, G, bias)
        if self._wkey is not None and all(
                np.array_equal(a, b) for a, b in zip(self._wkey, parts)):
            return self._wdev
        W2 = _pack_weights(U, V, C, G, bias)
        self._wdev = self._jax.device_put(W2, self._sharding)
        self._wkey = tuple(p.copy() for p in parts)
        return self._wdev

    def run(self, wdev, x0):
        """Returns the final f32 output [B, D] = x0 * dequant(u)."""
        jax = self._jax
        futs = []
        for k in range(NSLICE):
            rows = slice(k * ROWS, (k + 1) * ROWS)
            xq_k, sx_k = _quantize_x(x0[rows])
            ops = {"XQ": jax.device_put(xq_k, self._sharding),
                   "SX": jax.device_put(sx_k, self._sharding),
                   "WSH": wdev}
            args = tuple(ops[n] for n in self._arg_names)
            outs = self._jitted(*args, *self._zjit())
            byname = dict(zip(self._out_names, outs))
            futs.append((self._pool.submit(np.asarray, byname["UQ"]),
                         self._pool.submit(np.asarray, byname["US"])))
        out = np.empty((B, D), np.float32)
        for k, (fq, fs) in enumerate(futs):
            uq, us = fq.result(), fs.result()
            _decode_u(out, k * ROWS, uq, us, x0)
        return out


def _get_fast():
    if "fast" not in _CACHE:
        _CACHE["fast"] = _FastRunner()
    return _CACHE["fast"]


def _run_fallback(xq, sx, W2, x0):
    """Plain single-shot path via run_bass_kernel_spmd (full batch)."""
    from concourse.bass_utils import run_bass_kernel_spmd
    nc = _get_nc(BC)
    in_maps = []
    for c in range(NCORES):
        rows = slice(c * BC, (c + 1) * BC)
        in_maps.append({
            "XQ": xq[rows], "SX": sx[rows],
            "WSH": W2[c * WROWS_SH:(c + 1) * WROWS_SH],
        })
    res = run_bass_kernel_spmd(nc, in_maps, core_ids=list(range(NCORES)))
    out = np.empty((B, D), np.float32)
    for c in range(NCORES):
        rows = slice(c * BC, (c + 1) * BC)
        uq = res.results[c]["UQ"]
        us = res.results[c]["US"]
        np.multiply(uq, us.astype(np.float32), out=out[rows])
        np.multiply(out[rows], x0[rows], out=out[rows])
    return out


def kernel(inputs, U, V, C, G, bias):
    inputs = np.asarray(inputs, dtype=np.float32)
    U = np.asarray(U, dtype=np.float32)
    V = np.asarray(V, dtype=np.float32)
    C = np.asarray(C, dtype=np.float32)
    G = np.asarray(G, dtype=np.float32)
    bias = np.asarray(bias, dtype=np.float32)

    try:
        fast = _get_fast()
        return fast.run(fast.get_wdev(U, V, C, G, bias), inputs)
    except Exception:
        import sys, traceback
        traceback.print_exc()
        print("kernel: fast path failed; using fallback", file=sys.stderr)
        xq, sx = _quantize_x(inputs)
        W2 = _pack_weights(U, V, C, G, bias)
        return _run_fallback(xq, sx, W2, inputs)

